# revision 43
# baseline (speedup 1.0000x reference)
"""Trainium2 Bass kernel for nn_BlockMoEAdapters (8 NeuronCores, SPMD).

Sharding: tokens (B*N = 4096) split contiguously across 8 cores (512 each).
Cores 0-3 hold batch 0, cores 4-7 batch 1. Attention K/V are all-gathered
(fp8, split into two half-collectives issued right after their producing
GEMMs) within each 4-core batch group; MoE capacity ranks use a tiny 8-core
all-gather of per-core expert counts.

Speed notes (measured on HW, not the cost model): fp8 and bf16 matmuls both
run at 1 col/cycle; fp8e4 DoubleRow fuses 2 k-tiles per instruction at the
same throughput (halves instruction count). fp8 is still a big win for DMA
bytes, SBUF footprint, and the k/v collectives. The device fp8e4 is IEEE
e4m3 (bias 8, max 240) - hosts quantize with ml_dtypes.float8_e4m3.
Schedule: k+v are gathered in four quarter-collectives issued as soon as
each quarter's GEMM lands, so attention starts ~50us earlier; all small
constants are packed into 4 DMAs (tiny DMAs cost ~0.6us of queue time
each); the MoE capacity all-gather (32B, latency/skew-bound) is hidden by
splitting the output GEMM into a counts-independent mlp2 phase (accumulated
in-place over xres) and a tiny moe2+be2 phase afterwards; LN2 stats
accumulate inside the proj loop. Weight prescale 8x for the fp8 GEMMs
(dequant folded into exp scale / gelu scale / a proj-epilogue copy); we2 is
quantized unscaled so it can share the moe2 PSUM with bf16-free epilogues.
Softmax denominators ride as a ones-column in V (accumulated by the same
DR matmuls as ao) and are inverted with the fast approx reciprocal;
normalization is self-consistent with the fp8-quantized exp scores so fp8
attention error mostly cancels.
"""
import sys

for _p in ('/opt/trn_rl_repo',):
    if _p not in sys.path:
        sys.path.append(_p)

import ml_dtypes
import numpy as np

import concourse.bass as bass
import concourse.mybir as mybir
import concourse.tile as tile
from concourse import bacc
from concourse.bass_utils import run_bass_kernel_spmd

F32 = mybir.dt.float32
F32R = mybir.dt.float32r
BF16 = mybir.dt.bfloat16
F8 = mybir.dt.float8e4
AF = mybir.ActivationFunctionType
ALU = mybir.AluOpType
DR = mybir.MatmulPerfMode.DoubleRow

B, N, D = 2, 2048, 1024
H, HD = 16, 64
E, TOPK = 4, 2
MOEH, MLPH = 256, 4096
T = B * N
NC = 8
TL = T // NC          # 512 tokens per core
NT = TL // 128        # 4 token tiles
DT = D // 128         # 8 channel tiles
CAP = int(T * TOPK / E * 1.0)   # 2048
GRP = 4               # cores per kv-gather group
EPS = 1e-5
WS = 8.0              # fp8 weight prescale (dequant folded into epilogues)
SEXP = 0.125 / (WS * WS)   # exp scale: hd^-0.5 and two 8x weight scales

_cache = {}


def _mm(nc, out, lhsT, rhs, start, stop, dt=None):
    if dt is not None:
        lhsT, rhs = lhsT.bitcast(dt), rhs.bitcast(dt)
    nc.tensor.matmul(out, lhsT, rhs, start=start, stop=stop)


def _mmdr(nc, out, lhsT, rhs, start, stop):
    nc.tensor.matmul(out, lhsT, rhs, start=start, stop=stop, perf_mode=DR)


def _build():
    nc = bacc.Bacc("TRN2", target_bir_lowering=False, debug=False,
                   num_devices=NC)

    def din(name, shape, dt=F32):
        return nc.dram_tensor(name, list(shape), dt, kind="ExternalInput")

    xT_d = din("xT", (D, TL), BF16)
    # host-retiled weight slabs (see _prep_inputs for layouts)
    wqk_d = din("wqk_l", (128, 16 * DT * 128), F8)
    wv_d = din("wv_l", (128, 2 * DT * 512), F8)
    wproj_d = din("wproj_l", (128, DT * DT * 128), F8)
    wmlp1_d = din("wmlp1_l", (128, 32 * DT * 128), BF16)
    we1_d = din("we1_l", (128, 8 * DT * 128), F8)
    wout8_d = din("wout8_l", (128, DT * 8 * 128), F8)
    wm2_d = din("wm2_l", (128, DT * 32 * 128), BF16)
    cf32_d = din("cf32", (128, 472))         # packed f32 consts
    cbf_d = din("cbf", (128, 128), BF16)     # packed bf16 consts
    cb4_d = din("cb4", (E, 2048), BF16)      # gsel | be2 | noiseT
    crow_d = din("crow", (NC, 3))            # wpfx | broute | bnoise

    out_d = nc.dram_tensor("out", [D, TL], F32, kind="ExternalOutput")

    rg_kv = [[0, 1, 2, 3], [4, 5, 6, 7]]
    rg_all = [list(range(NC))]

    with tile.TileContext(nc) as tc:
        with (
            tc.tile_pool(name="dram", bufs=1, space="DRAM") as dpool,
            tc.tile_pool(name="consts", bufs=1) as cpool,
            tc.tile_pool(name="persist", bufs=1) as ppool,
            tc.tile_pool(name="ps_big", bufs=2, space="PSUM") as ps_big,
            tc.tile_pool(name="ps_bc", bufs=2, space="PSUM") as ps_bc,
            tc.tile_pool(name="ps_ao", bufs=2, space="PSUM") as ps_ao,
            tc.tile_pool(name="wslab", bufs=2) as wpool,
            tc.tile_pool(name="scratch", bufs=2) as spool,
        ):
            # ---------- collective bounce buffers (k+v packed, quarters) --
            KB = 256 * TL                 # k bytes per quarter (2 slabs)
            VB = 128 * 2 * 640            # v bytes per quarter (2 pairs)
            HB = KB + VB
            kv_in = [dpool.tile([1, HB], F8, name=f"kv_in{q_}")
                     for q_ in range(4)]
            kv_out = [dpool.tile([1, GRP * HB], F8, name=f"kv_out{q_}")
                      for q_ in range(4)]

            def kin_k(q):    # [256, TL] view of the k region
                return kv_in[q][0:1, 0:KB].rearrange(
                    "a (p t) -> (a p) t", t=TL)

            def kin_v(q):    # [128, 1280] view of the v region
                return kv_in[q][0:1, KB:HB].rearrange(
                    "a (p c) -> (a p) c", c=2 * 640)

            def kout_k(q, r):
                return kv_out[q][0:1, r * HB:r * HB + KB].rearrange(
                    "a (p t) -> (a p) t", t=TL)

            def kout_v(q, r):
                return kv_out[q][0:1, r * HB + KB:(r + 1) * HB].rearrange(
                    "a (p c) -> (a p) c", c=2 * 640)
            cnt_in = dpool.tile([1, E], F32, name="cnt_in")
            cnt_out = dpool.tile([NC, E], F32, name="cnt_out")

            # ---------- constants (4 packed DMAs on gpsimd) ----------
            cbf = cpool.tile([128, 128], BF16, tag="cbf", name="cbf")
            nc.gpsimd.dma_start(cbf[:], cbf_d[:])
            cf32 = cpool.tile([128, 472], F32, tag="cf32", name="cf32")
            nc.gpsimd.dma_start(cf32[:], cf32_d[:])
            cb4 = cpool.tile([E, 2048], BF16, tag="cb4", name="cb4")
            nc.gpsimd.dma_start(cb4[:], cb4_d[:])
            crow = cpool.tile([NC, 3], F32, tag="crow", name="crow")
            nc.gpsimd.dma_start(crow[:], crow_d[:])
            ln1g_sb = cf32[:, 0:8]
            ln1b_sb = cf32[:, 8:16]
            ln2g_sb = cf32[:, 16:24]
            ln2b_sb = cf32[:, 24:32]
            bproj_sb = cf32[:, 32:40]
            be1_sb = cf32[:, 40:48]
            bmlp1_sb = cf32[:, 48:80]
            bmlp2_sb = cf32[:, 80:88]
            ones_sb = cf32[:, 88:216]
            eye_sb = cf32[:, 216:344]
            utri_sb = cf32[:, 344:472]
            onesb_sb = cbf[:, 0:1]
            c116_sb = cbf[0:1, 0:64]
            wroute_sb = cbf[:, 64:96]
            wnoise_sb = cbf[:, 96:128]
            gsel_sb = cb4[:, 0:512]
            be2_sb = cb4[:, 512:1536]
            noiseT_sb = cb4[:, 1536:2048]
            wpfx_sb = crow[:, 0:1]
            broute_sb = crow[0:E, 1:2]
            bnoise_sb = crow[0:E, 2:3]

            # ---------- load x (CM, bf16; residual source) ----------
            xT_sb = []
            for j in range(DT):
                t = ppool.tile([128, TL], BF16, tag=f"xT{j}", name=f"xT{j}")
                qx = nc.sync if j % 2 == 0 else nc.scalar
                qx.dma_start(t[:], xT_d[j * 128:(j + 1) * 128, :])
                xT_sb.append(t)

            # channel-major fp8 activations as pair-tiles (k-tile pairs
            # adjacent inside each tile for DR; separate tiles keep the
            # dependency tracking fine-grained)
            def pair_tiles(tag):
                ts = [ppool.tile([128, 2 * TL], F8, tag=f"{tag}{u}",
                                 name=f"{tag}{u}") for u in range(DT // 2)]
                return ts, [t[:].rearrange("p (j t) -> p j t", t=TL)
                            for t in ts]

            x1T_t, x1T_p = pair_tiles("x1T")
            aoT_t, aoT_p = pair_tiles("aoT")
            x2T8_t, x2T8_p = pair_tiles("x2T8")
            Hg_t, Hg_p = pair_tiles("Hg")

            class PV:
                def __init__(self, parts):
                    self.parts = parts

                def __getitem__(self, idx):
                    # [:, j, :] or [:, 2u:2u+2, :] or [:, j, a:b]
                    _, js, ts_ = idx
                    if isinstance(js, slice):
                        u = js.start // 2
                        return self.parts[u][:, :, ts_]
                    return self.parts[js // 2][:, js % 2, ts_]

            x1T_v = PV(x1T_p)
            aoT_v = PV(aoT_p)
            x2T8_v = PV(x2T8_p)
            Hg_v = PV(Hg_p)

            # ---------- LayerNorm in CM (bf16 inputs) ----------
            def ln_stats_j(stat, xt, j):
                # accumulate mean/sq sums for channel tile j
                ones_col = onesb_sb[:, 0:1]
                _mm(nc, stat[0:1, :], ones_col, xt[:],
                    j == 0, j == DT - 1)
                sq = spool.tile([128, TL], BF16, tag="lnsq", name="lnsq",
                                bufs=2)
                nc.vector.tensor_tensor(sq[:], xt[:], xt[:], ALU.mult)
                _mm(nc, stat[32:33, :], ones_col, sq[:],
                    j == 0, j == DT - 1)

            def layernorm_cm(xtiles, g_sb, b_sb, wslice, stat=None):
                # wslice(j) -> destination AP for normalized tile j
                musum = stat[0:1, :]
                sqsum = stat[32:33, :]
                mu = spool.tile([1, TL], F32, tag="lnmu", name="lnmu", bufs=1)
                nc.vector.tensor_scalar_mul(mu[:], musum, 1.0 / D)
                msq = spool.tile([1, TL], F32, tag="lnscr", name="lnmsq",
                                 bufs=2)
                nc.vector.tensor_tensor(msq[:], mu[:], mu[:], ALU.mult)
                var = spool.tile([1, TL], F32, tag="lnscr", name="lnvar",
                                 bufs=2)
                nc.vector.scalar_tensor_tensor(var[:], sqsum, 1.0 / D,
                                               msq[:], ALU.mult, ALU.subtract)
                vare = spool.tile([1, TL], F32, tag="lnscr", name="lnvare",
                                  bufs=2)
                nc.vector.tensor_scalar_add(vare[:], var[:], EPS)
                sd = spool.tile([1, TL], F32, tag="lnscr", name="lnsd",
                                bufs=2)
                nc.scalar.activation(sd[:], vare[:], AF.Sqrt)
                rsig = spool.tile([1, TL], F32, tag="lnrsig", name="lnrsig",
                                  bufs=1)
                rscr = spool.tile([1, TL], F32, tag="lnrscr", name="lnrscr",
                                  bufs=1)
                nc.vector.reciprocal_approx_accurate(out=rsig[:], in_=sd[:],
                                                     scratch=rscr[:])
                mub_ps = ps_bc.tile([128, TL], F32, tag="bc", name="mub")
                _mm(nc, mub_ps[:], ones_sb[0:1, :], mu[:], True, True, F32)
                rsb_ps = ps_bc.tile([128, TL], F32, tag="bc", name="rsb")
                _mm(nc, rsb_ps[:], ones_sb[0:1, :], rsig[:], True, True, F32)
                mub = spool.tile([128, TL], F32, tag="mubsb", name="mubsb",
                                 bufs=1)
                nc.vector.tensor_copy(mub[:], mub_ps[:])
                rsb = spool.tile([128, TL], F32, tag="rsbsb", name="rsbsb",
                                 bufs=1)
                nc.vector.tensor_copy(rsb[:], rsb_ps[:])
                for j in range(DT):
                    t1 = spool.tile([128, TL], F32, tag="lnt1", name="lnt1",
                                    bufs=2)
                    nc.vector.tensor_tensor(t1[:], xtiles[j][:], mub[:],
                                            ALU.subtract)
                    t2 = spool.tile([128, TL], F32, tag="lnt2", name="lnt2",
                                    bufs=2)
                    nc.vector.tensor_tensor(t2[:], t1[:], rsb[:], ALU.mult)
                    nc.vector.tensor_scalar(wslice(j), t2[:], g_sb[:, j:j + 1],
                                            b_sb[:, j:j + 1], ALU.mult,
                                            ALU.add)

            qT_sb = [ppool.tile([128, TL], F8, tag=f"qT{m}", name=f"qT{m}")
                     for m in range(DT)]

            with tc.tile_pool(name="st1", bufs=2) as s1pool:
                stat1 = ps_ao.tile([33, TL], F32, tag="ao", name="lnstat1")
                for j in range(DT):
                    ln_stats_j(stat1, xT_sb[j], j)
                layernorm_cm(xT_sb, ln1g_sb, ln1b_sb,
                             lambda j: x1T_p[j // 2][:, j % 2, :],
                             stat=stat1)

                def qk_slab(m):
                    # one 128-out-ch slab of the q/k GEMM (fp8)
                    slab = wpool.tile([128, DT * 128], F8, tag="qkslab",
                                      name="qkslab", bufs=3)
                    nc.sync.dma_start(
                        slab[:], wqk_d[:, m * 1024:(m + 1) * 1024])
                    sv = slab[:].rearrange("p (k c) -> p k c", c=128)
                    ps = ps_big.tile([128, TL], F32, tag="big", name="qk")
                    for u in range(DT // 2):
                        _mmdr(nc, ps[:], sv[:, 2 * u:2 * u + 2, :],
                              x1T_v[:, 2 * u:2 * u + 2, :],
                              u == 0, u == DT // 2 - 1)
                    if m < DT:
                        nc.vector.tensor_copy(qT_sb[m][:], ps[:])
                    else:
                        ksb = s1pool.tile([128, TL], F8, tag="kevac",
                                          name="kevac", bufs=2)
                        nc.vector.tensor_copy(ksb[:], ps[:])
                        mk = m - DT
                        nc.sync.dma_start(
                            kin_k(mk // 2)[(mk % 2) * 128:(mk % 2 + 1) * 128,
                                           :], ksb[:])

                wv_sb = [None, None]

                def v_quarter(qq):
                    # 256 v-channels (2 pairs), TM orientation. pad layout
                    # [q(2), hh(2), mt(4), 80]: col 64 = ones so the ao
                    # matmul also accumulates softmax denominators.
                    nn, iq = qq // 2, qq % 2
                    if iq == 0:
                        wv_sb[nn] = s1pool.tile([128, DT * 512], F8,
                                                tag=f"wv{nn}", name="wv",
                                                bufs=1)
                        nc.gpsimd.dma_start(
                            wv_sb[nn][:],
                            wv_d[:, nn * 4096:(nn + 1) * 4096])
                    wv_v = wv_sb[nn][:].rearrange("p (k c) -> p k c", c=512)
                    vp = s1pool.tile([128, 2 * 640], F8, tag="vpad",
                                     name="vpad", bufs=2)
                    nc.vector.memset(vp[:], 1.0)
                    dst = vp[:].rearrange("p (q hh m c) -> p q hh m c",
                                          hh=2, m=NT, c=80)
                    for mt in range(NT):         # 4 token Mtiles
                        ps = ps_big.tile([128, 256], F32, tag="big",
                                         name="vps")
                        for kk in range(DT):
                            _mm(nc, ps[:],
                                x1T_v[:, kk, mt * 128:(mt + 1) * 128],
                                wv_v[:, kk, iq * 256:(iq + 1) * 256],
                                kk == 0, kk == DT - 1)
                        nc.vector.tensor_copy(
                            dst[:, :, :, mt, 0:64],
                            ps[:].rearrange("p (q hh c) -> p q hh c",
                                            hh=2, c=64))
                    nc.sync.dma_start(kin_v(qq), vp[:])

                def ag(src, dst):
                    nc.gpsimd.collective_compute(
                        "AllGather", ALU.bypass, replica_groups=rg_kv,
                        ins=[src[:].opt()], outs=[dst[:].opt()])

                # per quarter: 2 k slabs + v quarter -> packed AG; then q
                for qq in range(4):
                    qk_slab(DT + 2 * qq)
                    qk_slab(DT + 2 * qq + 1)
                    v_quarter(qq)
                    ag(kv_in[qq], kv_out[qq])
                for m in range(DT):
                    qk_slab(m)

            # ---------- preload all MoE first-layer weights (1 DMA) -------
            we1_all = cpool.tile([128, 8 * DT * 128], F8, tag="we1a",
                                 name="we1a")
            nc.gpsimd.dma_start(we1_all[:], we1_d[:])
            we1_v = we1_all[:].rearrange("p (m k c) -> p m k c", k=DT, c=128)

            # ---------- attention (2-head interleave, fp8, DR ao) ----------
            with (
                tc.tile_pool(name="attn", bufs=2) as apool,
                tc.tile_pool(name="vsb", bufs=2) as vpool,
                tc.tile_pool(name="ssb", bufs=4) as spool_s,
            ):
                for p in range(DT):              # head pair
                    hf, pq = p // 2, p % 2       # kv quarter, pair in qtr
                    kp = []
                    vt = []
                    for r in range(GRP):
                        kt_ = apool.tile([128, TL], F8, tag=f"kp{r}",
                                         name=f"kp{r}")
                        nc.sync.dma_start(
                            kt_[:],
                            kout_k(hf, r)[pq * 128:(pq + 1) * 128, :])
                        kp.append(kt_)
                        vt_ = vpool.tile([128, 640], F8, tag=f"vt{r}",
                                         name=f"vt{r}")
                        nc.sync.dma_start(
                            vt_[:],
                            kout_v(hf, r)[:, pq * 640:(pq + 1) * 640])
                        vt.append(vt_)
                    ao_ps = [ps_ao.tile([66, TL], F32, tag="ao",
                                        name=f"ao{hh}") for hh in range(2)]
                    for beat in range(8):        # 2 key tiles per beat
                        for hh in range(2):
                            po = 64 * hh
                            s_sb = spool_s.tile([128, 2 * TL], F8,
                                                tag="ssb", name="ssb")
                            s2 = ps_big.tile([128, 2 * TL], F32, tag="big",
                                             name="s2")
                            for u in range(2):
                                kt = 2 * beat + u
                                r, cc = kt // 4, kt % 4
                                _mm(nc, s2[:, u * TL:(u + 1) * TL],
                                    kp[r][po:po + 64,
                                          cc * 128:(cc + 1) * 128],
                                    qT_sb[p][po:po + 64, :], True, True)
                            nc.scalar.activation(s_sb[:], s2[:],
                                                 AF.Exp, scale=SEXP)
                            r, cc = (2 * beat) // 4, (2 * beat) % 4
                            vtv = vt[r][:].rearrange(
                                "p (hh m c) -> p hh m c", hh=2, c=80)
                            _mmdr(nc, ao_ps[hh][:],
                                  vtv[:, hh, cc:cc + 2, 0:66],
                                  s_sb[:].rearrange("p (u t) -> p u t",
                                                    t=TL),
                                  beat == 0, beat == 7)
                    for hh in range(2):
                        po = 64 * hh
                        dsb = spool_s.tile([1, TL], F32, tag="densb",
                                           name="densb", bufs=1)
                        nc.vector.tensor_copy(dsb[:], ao_ps[hh][64:65, :])
                        recip = spool_s.tile([1, TL], F32, tag="recip",
                                             name="recip", bufs=1)
                        nc.vector.reciprocal_approx_fast(
                            out=recip[:], in_=dsb[:])
                        recb = spool_s.tile([1, TL], BF16, tag="recb",
                                            name="recb", bufs=1)
                        nc.vector.tensor_copy(recb[:], recip[:])
                        bc_ps = ps_bc.tile([64, TL], F32, tag="bc",
                                           name="aobc")
                        _mm(nc, bc_ps[:], c116_sb, recb[:],
                            True, True)
                        bc_sb = spool_s.tile([64, TL], BF16, tag="aobcsb",
                                             name="aobcsb", bufs=2)
                        nc.vector.tensor_copy(bc_sb[:], bc_ps[:])
                        nc.vector.tensor_tensor(
                            aoT_p[p // 2][po:po + 64, p % 2, :],
                            ao_ps[hh][0:64, :], bc_sb[:], ALU.mult)

                # ---------- proj + residual ----------
                stat2 = ps_ao.tile([33, TL], F32, tag="ao", name="lnstat2")
                xres = []
                xresb = []
                for m in range(DT):
                    slab = wpool.tile([128, DT * 128], F8, tag="qkslab",
                                      name="projslab", bufs=3)
                    nc.sync.dma_start(
                        slab[:], wproj_d[:, m * 1024:(m + 1) * 1024])
                    sv = slab[:].rearrange("p (k c) -> p k c", c=128)
                    ps = ps_big.tile([128, TL], F32, tag="big", name="proj")
                    for u in range(DT // 2):
                        _mmdr(nc, ps[:], sv[:, 2 * u:2 * u + 2, :],
                              aoT_v[:, 2 * u:2 * u + 2, :],
                              u == 0, u == DT // 2 - 1)
                    pd = spool.tile([128, TL], F32, tag="projdq",
                                    name="projdq", bufs=2)
                    nc.scalar.activation(pd[:], ps[:], AF.Copy,
                                         scale=1.0 / (WS * WS))
                    xr = ppool.tile([128, TL], F32, tag=f"xres{m}",
                                    name=f"xres{m}")
                    nc.vector.scalar_tensor_tensor(
                        xr[:], pd[:], bproj_sb[:, m:m + 1], xT_sb[m][:],
                        ALU.add, ALU.add)
                    xres.append(xr)
                    xrb = ppool.tile([128, TL], BF16, tag=f"xresb{m}",
                                     name=f"xresb{m}")
                    nc.vector.tensor_copy(xrb[:], xr[:])
                    xresb.append(xrb)
                    ln_stats_j(stat2, xrb, m)

            # ---------- LN2 (bf16 out + fp8 copy) ----------
            x2T = [ppool.tile([128, TL], BF16, tag=f"x2T{j}", name=f"x2T{j}")
                   for j in range(DT)]
            layernorm_cm(xresb, ln2g_sb, ln2b_sb,
                         lambda j: x2T[j][:], stat=stat2)

            # ---------- router ----------
            rt_ps = ps_ao.tile([32 + E, TL], F32, tag="ao", name="rt")
            logit_ps = rt_ps[0:E, :]
            nlin_ps = rt_ps[32:32 + E, :]
            for j in range(DT):
                _mm(nc, logit_ps, wroute_sb[:, j * E:(j + 1) * E],
                    x2T[j][:], j == 0, j == DT - 1)
            for j in range(DT):
                _mm(nc, nlin_ps, wnoise_sb[:, j * E:(j + 1) * E],
                    x2T[j][:], j == 0, j == DT - 1)
            logits = spool.tile([E, TL], F32, tag="logits", name="logits",
                                bufs=1)
            nc.vector.tensor_scalar(logits[:], logit_ps,
                                    broute_sb[:, 0:1], None, ALU.add)
            spe = spool.tile([E, TL], BF16, tag="softpe", name="softpe",
                             bufs=1)
            nc.scalar.activation(spe[:], nlin_ps, AF.Exp,
                                 bias=bnoise_sb[:, 0:1])
            spe1 = spool.tile([E, TL], BF16, tag="softpe1", name="softpe1",
                              bufs=1)
            nc.vector.tensor_scalar_add(spe1[:], spe[:], 1.0)
            sp = spool.tile([E, TL], BF16, tag="softp", name="softp",
                            bufs=1)
            nc.scalar.activation(sp[:], spe1[:], AF.Ln)
            nsp = spool.tile([E, TL], BF16, tag="nsp", name="nsp", bufs=1)
            nc.vector.tensor_tensor(nsp[:], noiseT_sb, sp[:], ALU.mult)
            noisy_cm = spool.tile([E, TL], F32, tag="noisycm", name="noisycm",
                                  bufs=1)
            nc.vector.tensor_tensor(noisy_cm[:], nsp[:], logits[:], ALU.add)

            # ---------- top-2 gates (TM) ----------
            noisy8 = ppool.tile([128, 8 * NT], F32, tag="noisy8",
                                name="noisy8")
            nc.vector.memset(noisy8[:], -1e30)
            m8 = ppool.tile([128, 8 * NT], F32, tag="m8", name="m8")
            gate = ppool.tile([128, E * NT], F32, tag="gate", name="gate")
            mask = ppool.tile([128, E * NT], F32, tag="mask", name="mask")
            geT = ppool.tile([E, TL], BF16, tag="geT", name="geT")
            cnt_sb = ppool.tile([1, NT * E], F32, tag="cntsb", name="cntsb")
            for j in range(NT):
                tr_ps = ps_bc.tile([128, E], F32, tag="bc", name="ntr")
                nc.tensor.matmul(tr_ps[:],
                                 noisy_cm[:, j * 128:(j + 1) * 128],
                                 eye_sb[0:E, 0:E], is_transpose=True,
                                 start=True, stop=True)
                nc.vector.tensor_copy(noisy8[:, 8 * j:8 * j + E], tr_ps[:])
            for j in range(NT):
                nm = noisy8[:, 8 * j:8 * j + E]
                nc.vector.max(m8[:, 8 * j:8 * j + 8],
                              noisy8[:, 8 * j:8 * j + 8])
                v1 = m8[:, 8 * j:8 * j + 1]
                v2 = m8[:, 8 * j + 1:8 * j + 2]
                oh1 = spool.tile([128, E], F32, tag="oh1", name="oh1")
                nc.vector.tensor_scalar(oh1[:], nm, v1, None, ALU.is_ge)
                msk = mask[:, E * j:E * (j + 1)]
                nc.vector.tensor_scalar(msk, nm, v2, None, ALU.is_ge)
                oh2 = spool.tile([128, E], F32, tag="oh2", name="oh2")
                nc.vector.tensor_tensor(oh2[:], msk, oh1[:], ALU.subtract)
                negv1 = spool.tile([128, 1], F32, tag="negv1", name="negv1")
                nc.vector.tensor_scalar_mul(negv1[:], v1, -1.0)
                p2 = spool.tile([128, 1], F32, tag="p2", name="p2")
                nc.scalar.activation(p2[:], v2, AF.Exp, bias=negv1[:])
                dden = spool.tile([128, 1], F32, tag="dden", name="dden")
                nc.vector.tensor_scalar_add(dden[:], p2[:], 1.0)
                rd = spool.tile([128, 1], F32, tag="rd", name="rd")
                nc.vector.reciprocal(rd[:], dden[:])
                gnum = spool.tile([128, E], F32, tag="gnum", name="gnum")
                nc.vector.tensor_scalar(gnum[:], oh2[:], p2[:], None,
                                        ALU.mult)
                gnum2 = spool.tile([128, E], F32, tag="gnum2", name="gnum2")
                nc.vector.tensor_tensor(gnum2[:], gnum[:], oh1[:], ALU.add)
                nc.vector.tensor_scalar(gate[:, E * j:E * (j + 1)],
                                        gnum2[:], rd[:], None, ALU.mult)
                cps = ps_bc.tile([1, E], F32, tag="bc", name="cnt")
                _mm(nc, cps[:], ones_sb[:, 0:1], msk, True, True, F32)
                nc.vector.tensor_copy(cnt_sb[0:1, E * j:E * (j + 1)], cps[:])

            # total counts -> all-gather
            tot = spool.tile([1, E], F32, tag="cnttot", name="cnttot",
                             bufs=1)
            nc.vector.tensor_tensor(tot[:], cnt_sb[0:1, 0:E],
                                    cnt_sb[0:1, E:2 * E], ALU.add)
            nc.vector.tensor_tensor(tot[:], tot[:], cnt_sb[0:1, 2 * E:3 * E],
                                    ALU.add)
            nc.vector.tensor_tensor(tot[:], tot[:], cnt_sb[0:1, 3 * E:4 * E],
                                    ALU.add)
            nc.sync.dma_start(cnt_in[:], tot[:])

            # ---------- MLP hidden + MoE hidden (overlaps counts AG) ------
            for j in range(DT):
                nc.scalar.activation(x2T8_p[j // 2][:, j % 2, :],
                                     x2T[j][:], AF.Copy)
            Hmoe = []
            for me in range(2 * E):
                ps = ps_big.tile([128, TL], F32, tag="big", name="hmoe")
                for u in range(DT // 2):
                    _mmdr(nc, ps[:], we1_v[:, me, 2 * u:2 * u + 2, :],
                          x2T8_v[:, 2 * u:2 * u + 2, :],
                          u == 0, u == DT // 2 - 1)
                hs = ppool.tile([128, TL], BF16, tag=f"hmoe{me}",
                                name=f"hmoe{me}")
                nc.scalar.activation(
                    hs[:], ps[:], AF.Gelu, scale=1.0 / WS,
                    bias=be1_sb[:, me:me + 1])
                Hmoe.append(hs)

            Hm_sb = []
            for m in range(MLPH // 128):
                slab = wpool.tile([128, DT * 128], BF16, tag="m1slab",
                                  name="m1slab", bufs=4)
                q1 = nc.sync if m % 2 == 0 else nc.gpsimd
                q1.dma_start(
                    slab[:], wmlp1_d[:, m * 1024:(m + 1) * 1024])
                ps = ps_big.tile([128, TL], F32, tag="big", name="hm")
                for kk in range(DT):
                    _mm(nc, ps[:], slab[:, kk * 128:(kk + 1) * 128],
                        x2T[kk][:], kk == 0, kk == DT - 1)
                hm = ppool.tile([128, TL], BF16, tag=f"hm{m}", name=f"hm{m}")
                nc.scalar.activation(hm[:], ps[:], AF.Gelu,
                                     bias=bmlp1_sb[:, m:m + 1])
                Hm_sb.append(hm)
            nc.gpsimd.collective_compute(
                "AllGather", ALU.bypass, replica_groups=rg_all,
                ins=[cnt_in[:].opt()], outs=[cnt_out[:].opt()])

            # ---------- preout: mlp2 GEMM + bias + residual (cnt-free) --
            preout = []
            for m in range(DT):
                slab2 = wpool.tile([128, 32 * 128], BF16, tag="outslab",
                                   name="outslab")
                nc.sync.dma_start(
                    slab2[:], wm2_d[:, m * 4096:(m + 1) * 4096])
                ps = ps_big.tile([128, TL], F32, tag="big", name="pre")
                for kk in range(MLPH // 128):
                    _mm(nc, ps[:], slab2[:, kk * 128:(kk + 1) * 128],
                        Hm_sb[kk][:], kk == 0, kk == MLPH // 128 - 1)
                nc.vector.scalar_tensor_tensor(
                    xres[m][:], ps[:], bmlp2_sb[:, m:m + 1], xres[m][:],
                    ALU.add, ALU.add)
                preout.append(xres[m])

            # ---------- ranks / keep / gate_eff ----------
            cntg = spool.tile([NC, E], F32, tag="cntg", name="cntg", bufs=1)
            nc.sync.dma_start(cntg[:], cnt_out[:])
            off_ps = ps_bc.tile([1, E], F32, tag="bc", name="off")
            _mm(nc, off_ps[:], wpfx_sb, cntg[:], True, True, F32)
            car = spool.tile([1, E * NT], F32, tag="car", name="car", bufs=1)
            nc.vector.tensor_copy(car[:, 0:E], off_ps[:])
            for j in range(1, NT):
                nc.vector.tensor_tensor(car[:, E * j:E * (j + 1)],
                                        car[:, E * (j - 1):E * j],
                                        cnt_sb[0:1, E * (j - 1):E * j],
                                        ALU.add)
            ge_tm = ppool.tile([128, E * NT], F32, tag="getm", name="getm")
            for j in range(NT):
                rk_ps = ps_bc.tile([128, E], F32, tag="bc", name="rank")
                _mm(nc, rk_ps[:], utri_sb,
                    mask[:, E * j:E * (j + 1)], True, False, F32)
                _mm(nc, rk_ps[:], ones_sb[0:1, :],
                    car[:, E * j:E * (j + 1)], False, True, F32)
                keep = spool.tile([128, E], F32, tag="keep", name="keep")
                nc.vector.tensor_scalar(keep[:], rk_ps[:], float(CAP), None,
                                        ALU.is_lt)
                nc.vector.tensor_tensor(ge_tm[:, E * j:E * (j + 1)],
                                        gate[:, E * j:E * (j + 1)],
                                        keep[:], ALU.mult)
            for j in range(NT):
                tr_ps = ps_bc.tile([E, 128], F32, tag="bc", name="getr")
                nc.tensor.matmul(tr_ps[:], ge_tm[:, E * j:E * (j + 1)],
                                 eye_sb, is_transpose=True,
                                 start=True, stop=True)
                nc.vector.tensor_copy(geT[:, j * 128:(j + 1) * 128], tr_ps[:])

            # gate the MoE hidden
            for e in range(E):
                bc_ps = ps_bc.tile([128, TL], F32, tag="bc", name="gbc")
                _mm(nc, bc_ps[:], gsel_sb[:, e * 128:(e + 1) * 128],
                    geT[:], True, True)
                bc_sb = spool.tile([128, TL], BF16, tag="gbcsb", name="gbcsb",
                                   bufs=2)
                nc.vector.tensor_copy(bc_sb[:], bc_ps[:])
                for hmi in range(MOEH // 128):
                    me = 2 * e + hmi
                    nc.vector.tensor_tensor(Hg_p[me // 2][:, me % 2, :],
                                            Hmoe[me][:],
                                            bc_sb[:], ALU.mult)

            # ---------- output GEMM: moe(DR fp8) + be2, add preout --------
            for m in range(DT):
                slab8 = wpool.tile([128, 8 * 128], F8, tag="out8",
                                   name="out8")
                nc.gpsimd.dma_start(
                    slab8[:], wout8_d[:, m * 1024:(m + 1) * 1024])
                sv8 = slab8[:].rearrange("p (k c) -> p k c", c=128)
                ps = ps_big.tile([128, TL], F32, tag="big", name="out")
                for u in range(4):           # we2 DR pairs
                    _mmdr(nc, ps[:], sv8[:, 2 * u:2 * u + 2, :],
                          Hg_v[:, 2 * u:2 * u + 2, :], u == 0, False)
                _mm(nc, ps[:], be2_sb[:, m * 128:(m + 1) * 128],
                    geT[:], False, True)
                o = spool.tile([128, TL], F32, tag="outsb", name="outsb",
                               bufs=2)
                nc.vector.tensor_tensor(o[:], ps[:], preout[m][:], ALU.add)
                nc.sync.dma_start(out_d[m * 128:(m + 1) * 128, :], o[:])

    nc.compile()
    return nc


def _tile_lhst(w, n_k, n_m):
    # w: [n_k*128, n_m*128] -> [128, n_m, n_k, 128] -> [128, n_m*n_k*128]
    kdim, mdim = w.shape
    return np.ascontiguousarray(
        w.reshape(n_k, 128, n_m, 128).transpose(1, 2, 0, 3)
        .reshape(128, n_m * n_k * 128))


def _prep_inputs(inputs):
    f32 = lambda a: np.ascontiguousarray(np.asarray(a, np.float32))
    bf = lambda a: np.ascontiguousarray(
        np.asarray(a, np.float32).astype(ml_dtypes.bfloat16))
    f8 = lambda a, s=1.0: np.ascontiguousarray(
        (np.asarray(a, np.float32) * s).astype(ml_dtypes.float8_e4m3))
    x = f32(inputs["x"]).reshape(T, D)
    noise = f32(inputs["noise"]).reshape(T, E)
    w_qkv = np.asarray(inputs["w_qkv"], np.float32)
    wqkT = w_qkv[:2 * D].T                       # [D, 2048]
    wvT = w_qkv[2 * D:].T                        # [D, D]
    wprojT = np.asarray(inputs["w_proj"], np.float32).T
    we1 = np.asarray(inputs["we1"], np.float32)  # [E, D, MOEH]
    we2 = np.asarray(inputs["we2"], np.float32)  # [E, MOEH, D]
    wmlp1 = np.asarray(inputs["w_mlp1"], np.float32)   # [D, MLPH]
    wmlp2 = np.asarray(inputs["w_mlp2"], np.float32)   # [MLPH, D]

    # we1 slabs: m-index = e*2+hmi over [D, 256] each
    we1_flat = np.concatenate([we1[e] for e in range(E)], 1)  # [D, E*MOEH]
    # wout8: per m, 8 we2 tiles (e,hmi); wm2: per m, 32 wmlp2 tiles
    we2_l = we2.reshape(E, 2, 128, DT, 128).transpose(2, 3, 0, 1, 4) \
        .reshape(128, DT * 8 * 128)
    wm2_l = wmlp2.reshape(32, 128, DT, 128).transpose(1, 2, 0, 3) \
        .reshape(128, DT * 32 * 128)
    # wv: [128, nn, kk, 512]
    wv_l = wvT.reshape(DT, 128, 2, 512).transpose(1, 2, 0, 3) \
        .reshape(128, 2 * DT * 512)

    cols = lambda a, n: np.asarray(a, np.float32).reshape(n, 128).T
    cf32 = np.concatenate([
        cols(inputs["ln1_g"], DT), cols(inputs["ln1_b"], DT),
        cols(inputs["ln2_g"], DT), cols(inputs["ln2_b"], DT),
        cols(inputs["b_proj"], DT), cols(inputs["be1"], DT),
        cols(inputs["b_mlp1"], 32), cols(inputs["b_mlp2"], DT),
        np.ones((128, 128), np.float32),
        np.eye(128, dtype=np.float32),
        np.triu(np.ones((128, 128), np.float32), 1),
    ], 1)
    cbf = np.concatenate([
        np.ones((128, 64), np.float32),
        np.asarray(inputs["w_route"], np.float32).reshape(DT, 128, E)
        .transpose(1, 0, 2).reshape(128, DT * E),
        np.asarray(inputs["w_noise"], np.float32).reshape(DT, 128, E)
        .transpose(1, 0, 2).reshape(128, DT * E),
    ], 1)
    gsel = np.repeat(np.eye(E, dtype=np.float32), 128, 1)

    shared = dict(
        wqk_l=f8(_tile_lhst(wqkT, DT, 16), WS),
        wv_l=f8(wv_l, WS),
        wproj_l=f8(_tile_lhst(wprojT, DT, DT), WS),
        wmlp1_l=bf(_tile_lhst(wmlp1, DT, 32)),
        we1_l=f8(_tile_lhst(we1_flat, DT, 8), WS),
        wout8_l=f8(we2_l),
        wm2_l=bf(wm2_l),
        cf32=f32(cf32),
        cbf=bf(cbf),
    )
    in_maps = []
    for c in range(NC):
        m = dict(shared)
        m["xT"] = bf(x[c * TL:(c + 1) * TL].T)
        m["cb4"] = bf(np.concatenate([
            gsel, np.asarray(inputs["be2"], np.float32),
            noise[c * TL:(c + 1) * TL].T], 1))
        crow = np.zeros((NC, 3), np.float32)
        crow[:, 0] = (np.arange(NC) < c)
        crow[0:E, 1] = np.asarray(inputs["b_route"], np.float32)
        crow[0:E, 2] = np.asarray(inputs["b_noise"], np.float32)
        m["crow"] = crow
        in_maps.append(m)
    return in_maps


def _run(inputs, trace=False):
    if "nc" not in _cache:
        _cache["nc"] = _build()
    nc = _cache["nc"]
    in_maps = _prep_inputs(inputs)
    res = run_bass_kernel_spmd(nc, in_maps, core_ids=list(range(NC)),
                               trace=trace)
    _cache["last_res"] = res
    shards = [res.results[c]["out"] for c in range(NC)]   # each [D, TL]
    out = np.concatenate([np.asarray(s, np.float32).T for s in shards],
                         0).reshape(B, N, D)
    return out.astype(np.float32), res.exec_time_ns


def kernel(**inputs):
    out, _ = _run(inputs, trace=False)
    return out


# revision 44
# speedup vs baseline: 1.1217x; 1.1217x over previous
"""Trainium2 Bass kernel for nn_BlockMoEAdapters (8 NeuronCores, SPMD).

Sharding: tokens (B*N = 4096) split contiguously across 8 cores (512 each).
Cores 0-3 hold batch 0, cores 4-7 batch 1. Attention K/V are all-gathered
(fp8, split into two half-collectives issued right after their producing
GEMMs) within each 4-core batch group; MoE capacity ranks use a tiny 8-core
all-gather of per-core expert counts.

Speed notes (measured on HW, not the cost model): fp8 and bf16 matmuls both
run at 1 col/cycle; fp8e4 DoubleRow fuses 2 k-tiles per instruction at the
same throughput (halves instruction count). fp8 is still a big win for DMA
bytes, SBUF footprint, and the k/v collectives. The device fp8e4 is IEEE
e4m3 (bias 8, max 240) - hosts quantize with ml_dtypes.float8_e4m3.
Schedule: k+v are gathered in four quarter-collectives issued as soon as
each quarter's GEMM lands, so attention starts ~50us earlier; all small
constants are packed into 4 DMAs (tiny DMAs cost ~0.6us of queue time
each); the MoE capacity all-gather (32B, latency/skew-bound) is hidden by
splitting the output GEMM into a counts-independent mlp2 phase (accumulated
in-place over xres) and a tiny moe2+be2 phase afterwards; LN2 stats
accumulate inside the proj loop. Weight prescale 8x for the fp8 GEMMs
(dequant folded into exp scale / gelu scale / a proj-epilogue copy); we2 is
quantized unscaled so it can share the moe2 PSUM with bf16-free epilogues.
Softmax denominators ride as a ones-column in V (accumulated by the same
DR matmuls as ao) and are inverted with the fast approx reciprocal;
normalization is self-consistent with the fp8-quantized exp scores so fp8
attention error mostly cancels.
"""
import sys

for _p in ('/opt/trn_rl_repo',):
    if _p not in sys.path:
        sys.path.append(_p)

import ml_dtypes
import numpy as np

import concourse.bass as bass
import concourse.mybir as mybir
import concourse.tile as tile
from concourse import bacc
from concourse.bass_utils import run_bass_kernel_spmd

F32 = mybir.dt.float32
F32R = mybir.dt.float32r
BF16 = mybir.dt.bfloat16
F8 = mybir.dt.float8e4
AF = mybir.ActivationFunctionType
ALU = mybir.AluOpType
DR = mybir.MatmulPerfMode.DoubleRow

B, N, D = 2, 2048, 1024
H, HD = 16, 64
E, TOPK = 4, 2
MOEH, MLPH = 256, 4096
T = B * N
NC = 8
TL = T // NC          # 512 tokens per core
NT = TL // 128        # 4 token tiles
DT = D // 128         # 8 channel tiles
CAP = int(T * TOPK / E * 1.0)   # 2048
GRP = 4               # cores per kv-gather group
EPS = 1e-5
WS = 8.0              # fp8 weight prescale (dequant folded into epilogues)
SEXP = 0.125 / (WS * WS)   # exp scale: hd^-0.5 and two 8x weight scales

_cache = {}


def _mm(nc, out, lhsT, rhs, start, stop, dt=None):
    if dt is not None:
        lhsT, rhs = lhsT.bitcast(dt), rhs.bitcast(dt)
    nc.tensor.matmul(out, lhsT, rhs, start=start, stop=stop)


def _mmdr(nc, out, lhsT, rhs, start, stop):
    nc.tensor.matmul(out, lhsT, rhs, start=start, stop=stop, perf_mode=DR)


def _build():
    nc = bacc.Bacc("TRN2", target_bir_lowering=False, debug=False,
                   num_devices=NC)

    def din(name, shape, dt=F32):
        return nc.dram_tensor(name, list(shape), dt, kind="ExternalInput")

    xT_d = din("xT", (D, TL), BF16)
    # host-retiled weight slabs (see _prep_inputs for layouts)
    wqk_d = din("wqk_l", (128, 16 * DT * 128), F8)
    wv_d = din("wv_l", (128, 2 * DT * 512), F8)
    wproj_d = din("wproj_l", (128, DT * DT * 128), F8)
    wmlp1_d = din("wmlp1_l", (128, 32 * DT * 128), BF16)
    we1_d = din("we1_l", (128, 8 * DT * 128), F8)
    wout8_d = din("wout8_l", (128, DT * 8 * 128), F8)
    wm2_d = din("wm2_l", (128, DT * 32 * 128), BF16)
    cf32_d = din("cf32", (128, 472))         # packed f32 consts
    cbf_d = din("cbf", (128, 128), BF16)     # packed bf16 consts
    cb4_d = din("cb4", (E, 2048), BF16)      # gsel | be2 | noiseT
    crow_d = din("crow", (NC, 3))            # wpfx | broute | bnoise

    out_d = nc.dram_tensor("out", [D, TL], F32, kind="ExternalOutput")

    rg_kv = [[0, 1, 2, 3], [4, 5, 6, 7]]
    rg_all = [list(range(NC))]

    with tile.TileContext(nc) as tc:
        with (
            tc.tile_pool(name="dram", bufs=1, space="DRAM") as dpool,
            tc.tile_pool(name="consts", bufs=1) as cpool,
            tc.tile_pool(name="persist", bufs=1) as ppool,
            tc.tile_pool(name="ps_big", bufs=2, space="PSUM") as ps_big,
            tc.tile_pool(name="ps_bc", bufs=2, space="PSUM") as ps_bc,
            tc.tile_pool(name="ps_ao", bufs=2, space="PSUM") as ps_ao,
            tc.tile_pool(name="wslab", bufs=2) as wpool,
            tc.tile_pool(name="scratch", bufs=2) as spool,
        ):
            # ---------- collective bounce buffers (k+v packed, quarters) --
            KB = 256 * TL                 # k bytes per quarter (2 slabs)
            VB = 128 * 2 * 640            # v bytes per quarter (2 pairs)
            HB = KB + VB
            kv_in = [dpool.tile([1, HB], F8, name=f"kv_in{q_}")
                     for q_ in range(4)]
            kv_out = [dpool.tile([1, GRP * HB], F8, name=f"kv_out{q_}")
                      for q_ in range(4)]

            def kin_k(q):    # [256, TL] view of the k region
                return kv_in[q][0:1, 0:KB].rearrange(
                    "a (p t) -> (a p) t", t=TL)

            def kin_v(q):    # [128, 1280] view of the v region
                return kv_in[q][0:1, KB:HB].rearrange(
                    "a (p c) -> (a p) c", c=2 * 640)

            def kout_k(q, r):
                return kv_out[q][0:1, r * HB:r * HB + KB].rearrange(
                    "a (p t) -> (a p) t", t=TL)

            def kout_v(q, r):
                return kv_out[q][0:1, r * HB + KB:(r + 1) * HB].rearrange(
                    "a (p c) -> (a p) c", c=2 * 640)
            cnt_in = dpool.tile([1, E], F32, name="cnt_in")
            cnt_out = dpool.tile([NC, E], F32, name="cnt_out")

            # ---------- constants (4 packed DMAs on gpsimd) ----------
            cbf = cpool.tile([128, 128], BF16, tag="cbf", name="cbf")
            nc.gpsimd.dma_start(cbf[:], cbf_d[:])
            cf32 = cpool.tile([128, 472], F32, tag="cf32", name="cf32")
            nc.gpsimd.dma_start(cf32[:], cf32_d[:])
            cb4 = cpool.tile([E, 2048], BF16, tag="cb4", name="cb4")
            nc.gpsimd.dma_start(cb4[:], cb4_d[:])
            crow = cpool.tile([NC, 3], F32, tag="crow", name="crow")
            nc.gpsimd.dma_start(crow[:], crow_d[:])
            ln1g_sb = cf32[:, 0:8]
            ln1b_sb = cf32[:, 8:16]
            ln2g_sb = cf32[:, 16:24]
            ln2b_sb = cf32[:, 24:32]
            bproj_sb = cf32[:, 32:40]
            be1_sb = cf32[:, 40:48]
            bmlp1_sb = cf32[:, 48:80]
            bmlp2_sb = cf32[:, 80:88]
            ones_sb = cf32[:, 88:216]
            eye_sb = cf32[:, 216:344]
            utri_sb = cf32[:, 344:472]
            onesb_sb = cbf[:, 0:1]
            c116_sb = cbf[0:1, 0:64]
            wroute_sb = cbf[:, 64:96]
            wnoise_sb = cbf[:, 96:128]
            gsel_sb = cb4[:, 0:512]
            be2_sb = cb4[:, 512:1536]
            noiseT_sb = cb4[:, 1536:2048]
            wpfx_sb = crow[:, 0:1]
            broute_sb = crow[0:E, 1:2]
            bnoise_sb = crow[0:E, 2:3]

            # ---------- load x (CM, bf16; residual source) ----------
            xT_sb = []
            for j in range(DT):
                t = ppool.tile([128, TL], BF16, tag=f"xT{j}", name=f"xT{j}")
                qx = nc.sync if j % 2 == 0 else nc.scalar
                qx.dma_start(t[:], xT_d[j * 128:(j + 1) * 128, :])
                xT_sb.append(t)

            # channel-major fp8 activations as pair-tiles (k-tile pairs
            # adjacent inside each tile for DR; separate tiles keep the
            # dependency tracking fine-grained)
            def pair_tiles(tag):
                ts = [ppool.tile([128, 2 * TL], F8, tag=f"{tag}{u}",
                                 name=f"{tag}{u}") for u in range(DT // 2)]
                return ts, [t[:].rearrange("p (j t) -> p j t", t=TL)
                            for t in ts]

            x1T_t, x1T_p = pair_tiles("x1T")
            aoT_t, aoT_p = pair_tiles("aoT")
            x2T8_t, x2T8_p = pair_tiles("x2T8")
            Hg_t, Hg_p = pair_tiles("Hg")

            class PV:
                def __init__(self, parts):
                    self.parts = parts

                def __getitem__(self, idx):
                    # [:, j, :] or [:, 2u:2u+2, :] or [:, j, a:b]
                    _, js, ts_ = idx
                    if isinstance(js, slice):
                        u = js.start // 2
                        return self.parts[u][:, :, ts_]
                    return self.parts[js // 2][:, js % 2, ts_]

            x1T_v = PV(x1T_p)
            aoT_v = PV(aoT_p)
            x2T8_v = PV(x2T8_p)
            Hg_v = PV(Hg_p)

            # ---------- LayerNorm in CM (bf16 inputs) ----------
            def ln_stats_j(stat, xt, j):
                # accumulate mean/sq sums for channel tile j
                ones_col = onesb_sb[:, 0:1]
                _mm(nc, stat[0:1, :], ones_col, xt[:],
                    j == 0, j == DT - 1)
                sq = spool.tile([128, TL], BF16, tag="lnsq", name="lnsq",
                                bufs=2)
                nc.vector.tensor_tensor(sq[:], xt[:], xt[:], ALU.mult)
                _mm(nc, stat[32:33, :], ones_col, sq[:],
                    j == 0, j == DT - 1)

            def layernorm_cm(xtiles, g_sb, b_sb, wslice, stat=None):
                # wslice(j) -> destination AP for normalized tile j
                musum = stat[0:1, :]
                sqsum = stat[32:33, :]
                mu = spool.tile([1, TL], F32, tag="lnmu", name="lnmu", bufs=1)
                nc.vector.tensor_scalar_mul(mu[:], musum, 1.0 / D)
                msq = spool.tile([1, TL], F32, tag="lnscr", name="lnmsq",
                                 bufs=2)
                nc.vector.tensor_tensor(msq[:], mu[:], mu[:], ALU.mult)
                var = spool.tile([1, TL], F32, tag="lnscr", name="lnvar",
                                 bufs=2)
                nc.vector.scalar_tensor_tensor(var[:], sqsum, 1.0 / D,
                                               msq[:], ALU.mult, ALU.subtract)
                vare = spool.tile([1, TL], F32, tag="lnscr", name="lnvare",
                                  bufs=2)
                nc.vector.tensor_scalar_add(vare[:], var[:], EPS)
                sd = spool.tile([1, TL], F32, tag="lnscr", name="lnsd",
                                bufs=2)
                nc.scalar.activation(sd[:], vare[:], AF.Sqrt)
                rsig = spool.tile([1, TL], F32, tag="lnrsig", name="lnrsig",
                                  bufs=1)
                rscr = spool.tile([1, TL], F32, tag="lnrscr", name="lnrscr",
                                  bufs=1)
                nc.vector.reciprocal_approx_accurate(out=rsig[:], in_=sd[:],
                                                     scratch=rscr[:])
                mub_ps = ps_bc.tile([128, TL], F32, tag="bc", name="mub")
                _mm(nc, mub_ps[:], ones_sb[0:1, :], mu[:], True, True, F32)
                rsb_ps = ps_bc.tile([128, TL], F32, tag="bc", name="rsb")
                _mm(nc, rsb_ps[:], ones_sb[0:1, :], rsig[:], True, True, F32)
                mub = spool.tile([128, TL], F32, tag="mubsb", name="mubsb",
                                 bufs=1)
                nc.vector.tensor_copy(mub[:], mub_ps[:])
                rsb = spool.tile([128, TL], F32, tag="rsbsb", name="rsbsb",
                                 bufs=1)
                nc.vector.tensor_copy(rsb[:], rsb_ps[:])
                for j in range(DT):
                    t1 = spool.tile([128, TL], F32, tag="lnt1", name="lnt1",
                                    bufs=2)
                    nc.vector.tensor_tensor(t1[:], xtiles[j][:], mub[:],
                                            ALU.subtract)
                    t2 = spool.tile([128, TL], F32, tag="lnt2", name="lnt2",
                                    bufs=2)
                    nc.vector.tensor_tensor(t2[:], t1[:], rsb[:], ALU.mult)
                    nc.vector.tensor_scalar(wslice(j), t2[:], g_sb[:, j:j + 1],
                                            b_sb[:, j:j + 1], ALU.mult,
                                            ALU.add)

            qT_sb = [ppool.tile([128, TL], F8, tag=f"qT{m}", name=f"qT{m}")
                     for m in range(DT)]

            with tc.tile_pool(name="st1", bufs=2) as s1pool:
                stat1 = ps_ao.tile([33, TL], F32, tag="ao", name="lnstat1")
                for j in range(DT):
                    ln_stats_j(stat1, xT_sb[j], j)
                layernorm_cm(xT_sb, ln1g_sb, ln1b_sb,
                             lambda j: x1T_p[j // 2][:, j % 2, :],
                             stat=stat1)

                def qk_slab(m):
                    # one 128-out-ch slab of the q/k GEMM (fp8)
                    slab = wpool.tile([128, DT * 128], F8, tag="qkslab",
                                      name="qkslab", bufs=3)
                    nc.sync.dma_start(
                        slab[:], wqk_d[:, m * 1024:(m + 1) * 1024])
                    sv = slab[:].rearrange("p (k c) -> p k c", c=128)
                    ps = ps_big.tile([128, TL], F32, tag="big", name="qk")
                    for u in range(DT // 2):
                        _mmdr(nc, ps[:], sv[:, 2 * u:2 * u + 2, :],
                              x1T_v[:, 2 * u:2 * u + 2, :],
                              u == 0, u == DT // 2 - 1)
                    if m < DT:
                        nc.vector.tensor_copy(qT_sb[m][:], ps[:])
                    else:
                        ksb = s1pool.tile([128, TL], F8, tag="kevac",
                                          name="kevac", bufs=2)
                        nc.vector.tensor_copy(ksb[:], ps[:])
                        mk = m - DT
                        nc.sync.dma_start(
                            kin_k(mk // 2)[(mk % 2) * 128:(mk % 2 + 1) * 128,
                                           :], ksb[:])

                wv_sb = [None, None]

                def v_quarter(qq):
                    # 256 v-channels (2 pairs), TM orientation. pad layout
                    # [q(2), hh(2), mt(4), 80]: col 64 = ones so the ao
                    # matmul also accumulates softmax denominators.
                    nn, iq = qq // 2, qq % 2
                    if iq == 0:
                        wv_sb[nn] = s1pool.tile([128, DT * 512], F8,
                                                tag=f"wv{nn}", name="wv",
                                                bufs=1)
                        nc.gpsimd.dma_start(
                            wv_sb[nn][:],
                            wv_d[:, nn * 4096:(nn + 1) * 4096])
                    wv_v = wv_sb[nn][:].rearrange("p (k c) -> p k c", c=512)
                    vp = s1pool.tile([128, 2 * 640], F8, tag="vpad",
                                     name="vpad", bufs=2)
                    nc.vector.memset(vp[:], 1.0)
                    dst = vp[:].rearrange("p (q hh m c) -> p q hh m c",
                                          hh=2, m=NT, c=80)
                    for mt in range(NT):         # 4 token Mtiles
                        ps = ps_big.tile([128, 256], F32, tag="big",
                                         name="vps")
                        for kk in range(DT):
                            _mm(nc, ps[:],
                                x1T_v[:, kk, mt * 128:(mt + 1) * 128],
                                wv_v[:, kk, iq * 256:(iq + 1) * 256],
                                kk == 0, kk == DT - 1)
                        nc.vector.tensor_copy(
                            dst[:, :, :, mt, 0:64],
                            ps[:].rearrange("p (q hh c) -> p q hh c",
                                            hh=2, c=64))
                    nc.sync.dma_start(kin_v(qq), vp[:])

                def ag(src, dst):
                    nc.gpsimd.collective_compute(
                        "AllGather", ALU.bypass, replica_groups=rg_kv,
                        ins=[src[:].opt()], outs=[dst[:].opt()])

                # per quarter: 2 k slabs + v quarter -> packed AG; then q
                for qq in range(4):
                    qk_slab(DT + 2 * qq)
                    qk_slab(DT + 2 * qq + 1)
                    v_quarter(qq)
                    ag(kv_in[qq], kv_out[qq])
                for m in range(DT):
                    qk_slab(m)

            # ---------- preload all MoE first-layer weights (1 DMA) -------
            we1_all = cpool.tile([128, 8 * DT * 128], F8, tag="we1a",
                                 name="we1a")
            nc.gpsimd.dma_start(we1_all[:], we1_d[:])
            we1_v = we1_all[:].rearrange("p (m k c) -> p m k c", k=DT, c=128)

            # ---------- attention (2-head interleave, fp8, DR ao) ----------
            with (
                tc.tile_pool(name="attn", bufs=2) as apool,
                tc.tile_pool(name="vsb", bufs=2) as vpool,
                tc.tile_pool(name="ssb", bufs=4) as spool_s,
            ):
                for p in range(DT):              # head pair
                    hf, pq = p // 2, p % 2       # kv quarter, pair in qtr
                    kp = []
                    vt = []
                    for r in range(GRP):
                        kt_ = apool.tile([128, TL], F8, tag=f"kp{r}",
                                         name=f"kp{r}")
                        nc.sync.dma_start(
                            kt_[:],
                            kout_k(hf, r)[pq * 128:(pq + 1) * 128, :])
                        kp.append(kt_)
                        vt_ = vpool.tile([128, 640], F8, tag=f"vt{r}",
                                         name=f"vt{r}")
                        nc.sync.dma_start(
                            vt_[:],
                            kout_v(hf, r)[:, pq * 640:(pq + 1) * 640])
                        vt.append(vt_)
                    ao_ps = [ps_ao.tile([66, TL], F32, tag="ao",
                                        name=f"ao{hh}") for hh in range(2)]
                    for beat in range(8):        # 2 key tiles per beat
                        for hh in range(2):
                            po = 64 * hh
                            s_sb = spool_s.tile([128, 2 * TL], F8,
                                                tag="ssb", name="ssb")
                            s2 = ps_big.tile([128, 2 * TL], F32, tag="big",
                                             name="s2")
                            for u in range(2):
                                kt = 2 * beat + u
                                r, cc = kt // 4, kt % 4
                                _mm(nc, s2[:, u * TL:(u + 1) * TL],
                                    kp[r][po:po + 64,
                                          cc * 128:(cc + 1) * 128],
                                    qT_sb[p][po:po + 64, :], True, True)
                            nc.scalar.activation(s_sb[:], s2[:],
                                                 AF.Exp, scale=SEXP)
                            r, cc = (2 * beat) // 4, (2 * beat) % 4
                            vtv = vt[r][:].rearrange(
                                "p (hh m c) -> p hh m c", hh=2, c=80)
                            _mmdr(nc, ao_ps[hh][:],
                                  vtv[:, hh, cc:cc + 2, 0:66],
                                  s_sb[:].rearrange("p (u t) -> p u t",
                                                    t=TL),
                                  beat == 0, beat == 7)
                    for hh in range(2):
                        po = 64 * hh
                        dsb = spool_s.tile([1, TL], F32, tag="densb",
                                           name="densb", bufs=1)
                        nc.vector.tensor_copy(dsb[:], ao_ps[hh][64:65, :])
                        recip = spool_s.tile([1, TL], F32, tag="recip",
                                             name="recip", bufs=1)
                        nc.vector.reciprocal_approx_fast(
                            out=recip[:], in_=dsb[:])
                        recb = spool_s.tile([1, TL], BF16, tag="recb",
                                            name="recb", bufs=1)
                        nc.vector.tensor_copy(recb[:], recip[:])
                        bc_ps = ps_bc.tile([64, TL], F32, tag="bc",
                                           name="aobc")
                        _mm(nc, bc_ps[:], c116_sb, recb[:],
                            True, True)
                        bc_sb = spool_s.tile([64, TL], BF16, tag="aobcsb",
                                             name="aobcsb", bufs=2)
                        nc.vector.tensor_copy(bc_sb[:], bc_ps[:])
                        nc.vector.tensor_tensor(
                            aoT_p[p // 2][po:po + 64, p % 2, :],
                            ao_ps[hh][0:64, :], bc_sb[:], ALU.mult)

                # ---------- proj + residual ----------
                stat2 = ps_ao.tile([33, TL], F32, tag="ao", name="lnstat2")
                xres = []
                xresb = []
                for m in range(DT):
                    slab = wpool.tile([128, DT * 128], F8, tag="qkslab",
                                      name="projslab", bufs=3)
                    nc.sync.dma_start(
                        slab[:], wproj_d[:, m * 1024:(m + 1) * 1024])
                    sv = slab[:].rearrange("p (k c) -> p k c", c=128)
                    ps = ps_big.tile([128, TL], F32, tag="big", name="proj")
                    for u in range(DT // 2):
                        _mmdr(nc, ps[:], sv[:, 2 * u:2 * u + 2, :],
                              aoT_v[:, 2 * u:2 * u + 2, :],
                              u == 0, u == DT // 2 - 1)
                    pd = spool.tile([128, TL], F32, tag="projdq",
                                    name="projdq", bufs=2)
                    nc.scalar.activation(pd[:], ps[:], AF.Copy,
                                         scale=1.0 / (WS * WS))
                    xr = ppool.tile([128, TL], F32, tag=f"xres{m}",
                                    name=f"xres{m}")
                    nc.vector.scalar_tensor_tensor(
                        xr[:], pd[:], bproj_sb[:, m:m + 1], xT_sb[m][:],
                        ALU.add, ALU.add)
                    xres.append(xr)
                    xrb = ppool.tile([128, TL], BF16, tag=f"xresb{m}",
                                     name=f"xresb{m}")
                    nc.vector.tensor_copy(xrb[:], xr[:])
                    xresb.append(xrb)
                    ln_stats_j(stat2, xrb, m)

            # ---------- LN2 (bf16 out + fp8 copy) ----------
            x2T = [ppool.tile([128, TL], BF16, tag=f"x2T{j}", name=f"x2T{j}")
                   for j in range(DT)]
            layernorm_cm(xresb, ln2g_sb, ln2b_sb,
                         lambda j: x2T[j][:], stat=stat2)

            # ---------- router ----------
            rt_ps = ps_ao.tile([32 + E, TL], F32, tag="ao", name="rt")
            logit_ps = rt_ps[0:E, :]
            nlin_ps = rt_ps[32:32 + E, :]
            for j in range(DT):
                _mm(nc, logit_ps, wroute_sb[:, j * E:(j + 1) * E],
                    x2T[j][:], j == 0, j == DT - 1)
            for j in range(DT):
                _mm(nc, nlin_ps, wnoise_sb[:, j * E:(j + 1) * E],
                    x2T[j][:], j == 0, j == DT - 1)
            logits = spool.tile([E, TL], F32, tag="logits", name="logits",
                                bufs=1)
            nc.vector.tensor_scalar(logits[:], logit_ps,
                                    broute_sb[:, 0:1], None, ALU.add)
            spe = spool.tile([E, TL], BF16, tag="softpe", name="softpe",
                             bufs=1)
            nc.scalar.activation(spe[:], nlin_ps, AF.Exp,
                                 bias=bnoise_sb[:, 0:1])
            spe1 = spool.tile([E, TL], BF16, tag="softpe1", name="softpe1",
                              bufs=1)
            nc.vector.tensor_scalar_add(spe1[:], spe[:], 1.0)
            sp = spool.tile([E, TL], BF16, tag="softp", name="softp",
                            bufs=1)
            nc.scalar.activation(sp[:], spe1[:], AF.Ln)
            nsp = spool.tile([E, TL], BF16, tag="nsp", name="nsp", bufs=1)
            nc.vector.tensor_tensor(nsp[:], noiseT_sb, sp[:], ALU.mult)
            noisy_cm = spool.tile([E, TL], F32, tag="noisycm", name="noisycm",
                                  bufs=1)
            nc.vector.tensor_tensor(noisy_cm[:], nsp[:], logits[:], ALU.add)

            # ---------- top-2 gates (TM) ----------
            noisy8 = ppool.tile([128, 8 * NT], F32, tag="noisy8",
                                name="noisy8")
            nc.vector.memset(noisy8[:], -1e30)
            m8 = ppool.tile([128, 8 * NT], F32, tag="m8", name="m8")
            gate = ppool.tile([128, E * NT], F32, tag="gate", name="gate")
            mask = ppool.tile([128, E * NT], F32, tag="mask", name="mask")
            geT = ppool.tile([E, TL], BF16, tag="geT", name="geT")
            cnt_sb = ppool.tile([1, NT * E], F32, tag="cntsb", name="cntsb")
            for j in range(NT):
                tr_ps = ps_bc.tile([128, E], F32, tag="bc", name="ntr")
                nc.tensor.matmul(tr_ps[:],
                                 noisy_cm[:, j * 128:(j + 1) * 128],
                                 eye_sb[0:E, 0:E], is_transpose=True,
                                 start=True, stop=True)
                nc.vector.tensor_copy(noisy8[:, 8 * j:8 * j + E], tr_ps[:])
            dv = spool.tile([128, NT], F32, tag="dv", name="dv")
            for j in range(NT):
                nc.vector.max(m8[:, 8 * j:8 * j + 8],
                              noisy8[:, 8 * j:8 * j + 8])
                nc.vector.tensor_tensor(dv[:, j:j + 1],
                                        m8[:, 8 * j + 1:8 * j + 2],
                                        m8[:, 8 * j:8 * j + 1],
                                        ALU.subtract)
            p2a = spool.tile([128, NT], F32, tag="p2a", name="p2a")
            nc.scalar.activation(p2a[:], dv[:], AF.Exp)
            dden = spool.tile([128, NT], F32, tag="dden", name="dden")
            nc.vector.tensor_scalar_add(dden[:], p2a[:], 1.0)
            rda = spool.tile([128, NT], F32, tag="rda", name="rda")
            nc.vector.reciprocal(rda[:], dden[:])
            for j in range(NT):
                nm = noisy8[:, 8 * j:8 * j + E]
                v1 = m8[:, 8 * j:8 * j + 1]
                v2 = m8[:, 8 * j + 1:8 * j + 2]
                oh1 = spool.tile([128, E], F32, tag="oh1", name="oh1")
                nc.vector.tensor_scalar(oh1[:], nm, v1, None, ALU.is_ge)
                msk = mask[:, E * j:E * (j + 1)]
                nc.vector.tensor_scalar(msk, nm, v2, None, ALU.is_ge)
                oh2 = spool.tile([128, E], F32, tag="oh2", name="oh2")
                nc.vector.tensor_tensor(oh2[:], msk, oh1[:], ALU.subtract)
                gnum = spool.tile([128, E], F32, tag="gnum", name="gnum")
                nc.vector.tensor_scalar(gnum[:], oh2[:], p2a[:, j:j + 1],
                                        None, ALU.mult)
                gnum2 = spool.tile([128, E], F32, tag="gnum2", name="gnum2")
                nc.vector.tensor_tensor(gnum2[:], gnum[:], oh1[:], ALU.add)
                nc.vector.tensor_scalar(gate[:, E * j:E * (j + 1)],
                                        gnum2[:], rda[:, j:j + 1], None,
                                        ALU.mult)
                cps = ps_bc.tile([1, E], F32, tag="bc", name="cnt")
                _mm(nc, cps[:], ones_sb[:, 0:1], msk, True, True, F32)
                nc.vector.tensor_copy(cnt_sb[0:1, E * j:E * (j + 1)], cps[:])

            # total counts -> all-gather
            tot = spool.tile([1, E], F32, tag="cnttot", name="cnttot",
                             bufs=1)
            nc.vector.tensor_tensor(tot[:], cnt_sb[0:1, 0:E],
                                    cnt_sb[0:1, E:2 * E], ALU.add)
            nc.vector.tensor_tensor(tot[:], tot[:], cnt_sb[0:1, 2 * E:3 * E],
                                    ALU.add)
            nc.vector.tensor_tensor(tot[:], tot[:], cnt_sb[0:1, 3 * E:4 * E],
                                    ALU.add)
            nc.sync.dma_start(cnt_in[:], tot[:])

            # ---------- MLP hidden + MoE hidden (overlaps counts AG) ------
            for j in range(DT):
                nc.scalar.activation(x2T8_p[j // 2][:, j % 2, :],
                                     x2T[j][:], AF.Copy)
            Hmoe = []
            for me in range(2 * E):
                ps = ps_big.tile([128, TL], F32, tag="big", name="hmoe")
                for u in range(DT // 2):
                    _mmdr(nc, ps[:], we1_v[:, me, 2 * u:2 * u + 2, :],
                          x2T8_v[:, 2 * u:2 * u + 2, :],
                          u == 0, u == DT // 2 - 1)
                hs = ppool.tile([128, TL], BF16, tag=f"hmoe{me}",
                                name=f"hmoe{me}")
                nc.scalar.activation(
                    hs[:], ps[:], AF.Gelu, scale=1.0 / WS,
                    bias=be1_sb[:, me:me + 1])
                Hmoe.append(hs)

            Hm_sb = []
            for m in range(MLPH // 128):
                slab = wpool.tile([128, DT * 128], BF16, tag="m1slab",
                                  name="m1slab", bufs=4)
                q1 = nc.sync if m % 2 == 0 else nc.gpsimd
                q1.dma_start(
                    slab[:], wmlp1_d[:, m * 1024:(m + 1) * 1024])
                ps = ps_big.tile([128, TL], F32, tag="big", name="hm")
                for kk in range(DT):
                    _mm(nc, ps[:], slab[:, kk * 128:(kk + 1) * 128],
                        x2T[kk][:], kk == 0, kk == DT - 1)
                hm = ppool.tile([128, TL], BF16, tag=f"hm{m}", name=f"hm{m}")
                nc.scalar.activation(hm[:], ps[:], AF.Gelu,
                                     bias=bmlp1_sb[:, m:m + 1])
                Hm_sb.append(hm)
            nc.gpsimd.collective_compute(
                "AllGather", ALU.bypass, replica_groups=rg_all,
                ins=[cnt_in[:].opt()], outs=[cnt_out[:].opt()])

            # ---------- preout: mlp2 GEMM + bias + residual (cnt-free) --
            preout = []
            for m in range(DT):
                slab2 = wpool.tile([128, 32 * 128], BF16, tag="outslab",
                                   name="outslab")
                nc.sync.dma_start(
                    slab2[:], wm2_d[:, m * 4096:(m + 1) * 4096])
                ps = ps_big.tile([128, TL], F32, tag="big", name="pre")
                for kk in range(MLPH // 128):
                    _mm(nc, ps[:], slab2[:, kk * 128:(kk + 1) * 128],
                        Hm_sb[kk][:], kk == 0, kk == MLPH // 128 - 1)
                nc.vector.scalar_tensor_tensor(
                    xres[m][:], ps[:], bmlp2_sb[:, m:m + 1], xres[m][:],
                    ALU.add, ALU.add)
                preout.append(xres[m])

            # ---------- ranks / keep / gate_eff ----------
            cntg = spool.tile([NC, E], F32, tag="cntg", name="cntg", bufs=1)
            nc.sync.dma_start(cntg[:], cnt_out[:])
            off_ps = ps_bc.tile([1, E], F32, tag="bc", name="off")
            _mm(nc, off_ps[:], wpfx_sb, cntg[:], True, True, F32)
            car = spool.tile([1, E * NT], F32, tag="car", name="car", bufs=1)
            nc.vector.tensor_copy(car[:, 0:E], off_ps[:])
            for j in range(1, NT):
                nc.vector.tensor_tensor(car[:, E * j:E * (j + 1)],
                                        car[:, E * (j - 1):E * j],
                                        cnt_sb[0:1, E * (j - 1):E * j],
                                        ALU.add)
            ge_tm = ppool.tile([128, E * NT], F32, tag="getm", name="getm")
            for j in range(NT):
                rk_ps = ps_bc.tile([128, E], F32, tag="bc", name="rank")
                _mm(nc, rk_ps[:], utri_sb,
                    mask[:, E * j:E * (j + 1)], True, False, F32)
                _mm(nc, rk_ps[:], ones_sb[0:1, :],
                    car[:, E * j:E * (j + 1)], False, True, F32)
                keep = spool.tile([128, E], F32, tag="keep", name="keep")
                nc.vector.tensor_scalar(keep[:], rk_ps[:], float(CAP), None,
                                        ALU.is_lt)
                nc.vector.tensor_tensor(ge_tm[:, E * j:E * (j + 1)],
                                        gate[:, E * j:E * (j + 1)],
                                        keep[:], ALU.mult)
            for j in range(NT):
                tr_ps = ps_bc.tile([E, 128], F32, tag="bc", name="getr")
                nc.tensor.matmul(tr_ps[:], ge_tm[:, E * j:E * (j + 1)],
                                 eye_sb, is_transpose=True,
                                 start=True, stop=True)
                nc.vector.tensor_copy(geT[:, j * 128:(j + 1) * 128], tr_ps[:])

            # gate the MoE hidden
            for e in range(E):
                bc_ps = ps_bc.tile([128, TL], F32, tag="bc", name="gbc")
                _mm(nc, bc_ps[:], gsel_sb[:, e * 128:(e + 1) * 128],
                    geT[:], True, True)
                bc_sb = spool.tile([128, TL], BF16, tag="gbcsb", name="gbcsb",
                                   bufs=2)
                nc.vector.tensor_copy(bc_sb[:], bc_ps[:])
                for hmi in range(MOEH // 128):
                    me = 2 * e + hmi
                    nc.vector.tensor_tensor(Hg_p[me // 2][:, me % 2, :],
                                            Hmoe[me][:],
                                            bc_sb[:], ALU.mult)

            # ---------- output GEMM: moe(DR fp8) + be2, add preout --------
            for m in range(DT):
                slab8 = wpool.tile([128, 8 * 128], F8, tag="out8",
                                   name="out8")
                nc.gpsimd.dma_start(
                    slab8[:], wout8_d[:, m * 1024:(m + 1) * 1024])
                sv8 = slab8[:].rearrange("p (k c) -> p k c", c=128)
                ps = ps_big.tile([128, TL], F32, tag="big", name="out")
                for u in range(4):           # we2 DR pairs
                    _mmdr(nc, ps[:], sv8[:, 2 * u:2 * u + 2, :],
                          Hg_v[:, 2 * u:2 * u + 2, :], u == 0, False)
                _mm(nc, ps[:], be2_sb[:, m * 128:(m + 1) * 128],
                    geT[:], False, True)
                o = spool.tile([128, TL], F32, tag="outsb", name="outsb",
                               bufs=2)
                nc.vector.tensor_tensor(o[:], ps[:], preout[m][:], ALU.add)
                nc.sync.dma_start(out_d[m * 128:(m + 1) * 128, :], o[:])

    nc.compile()
    return nc


def _tile_lhst(w, n_k, n_m):
    # w: [n_k*128, n_m*128] -> [128, n_m, n_k, 128] -> [128, n_m*n_k*128]
    kdim, mdim = w.shape
    return np.ascontiguousarray(
        w.reshape(n_k, 128, n_m, 128).transpose(1, 2, 0, 3)
        .reshape(128, n_m * n_k * 128))


def _prep_inputs(inputs):
    f32 = lambda a: np.ascontiguousarray(np.asarray(a, np.float32))
    bf = lambda a: np.ascontiguousarray(
        np.asarray(a, np.float32).astype(ml_dtypes.bfloat16))
    f8 = lambda a, s=1.0: np.ascontiguousarray(
        (np.asarray(a, np.float32) * s).astype(ml_dtypes.float8_e4m3))
    x = f32(inputs["x"]).reshape(T, D)
    noise = f32(inputs["noise"]).reshape(T, E)
    w_qkv = np.asarray(inputs["w_qkv"], np.float32)
    wqkT = w_qkv[:2 * D].T                       # [D, 2048]
    wvT = w_qkv[2 * D:].T                        # [D, D]
    wprojT = np.asarray(inputs["w_proj"], np.float32).T
    we1 = np.asarray(inputs["we1"], np.float32)  # [E, D, MOEH]
    we2 = np.asarray(inputs["we2"], np.float32)  # [E, MOEH, D]
    wmlp1 = np.asarray(inputs["w_mlp1"], np.float32)   # [D, MLPH]
    wmlp2 = np.asarray(inputs["w_mlp2"], np.float32)   # [MLPH, D]

    # we1 slabs: m-index = e*2+hmi over [D, 256] each
    we1_flat = np.concatenate([we1[e] for e in range(E)], 1)  # [D, E*MOEH]
    # wout8: per m, 8 we2 tiles (e,hmi); wm2: per m, 32 wmlp2 tiles
    we2_l = we2.reshape(E, 2, 128, DT, 128).transpose(2, 3, 0, 1, 4) \
        .reshape(128, DT * 8 * 128)
    wm2_l = wmlp2.reshape(32, 128, DT, 128).transpose(1, 2, 0, 3) \
        .reshape(128, DT * 32 * 128)
    # wv: [128, nn, kk, 512]
    wv_l = wvT.reshape(DT, 128, 2, 512).transpose(1, 2, 0, 3) \
        .reshape(128, 2 * DT * 512)

    cols = lambda a, n: np.asarray(a, np.float32).reshape(n, 128).T
    cf32 = np.concatenate([
        cols(inputs["ln1_g"], DT), cols(inputs["ln1_b"], DT),
        cols(inputs["ln2_g"], DT), cols(inputs["ln2_b"], DT),
        cols(inputs["b_proj"], DT), cols(inputs["be1"], DT),
        cols(inputs["b_mlp1"], 32), cols(inputs["b_mlp2"], DT),
        np.ones((128, 128), np.float32),
        np.eye(128, dtype=np.float32),
        np.triu(np.ones((128, 128), np.float32), 1),
    ], 1)
    cbf = np.concatenate([
        np.ones((128, 64), np.float32),
        np.asarray(inputs["w_route"], np.float32).reshape(DT, 128, E)
        .transpose(1, 0, 2).reshape(128, DT * E),
        np.asarray(inputs["w_noise"], np.float32).reshape(DT, 128, E)
        .transpose(1, 0, 2).reshape(128, DT * E),
    ], 1)
    gsel = np.repeat(np.eye(E, dtype=np.float32), 128, 1)

    shared = dict(
        wqk_l=f8(_tile_lhst(wqkT, DT, 16), WS),
        wv_l=f8(wv_l, WS),
        wproj_l=f8(_tile_lhst(wprojT, DT, DT), WS),
        wmlp1_l=bf(_tile_lhst(wmlp1, DT, 32)),
        we1_l=f8(_tile_lhst(we1_flat, DT, 8), WS),
        wout8_l=f8(we2_l),
        wm2_l=bf(wm2_l),
        cf32=f32(cf32),
        cbf=bf(cbf),
    )
    in_maps = []
    for c in range(NC):
        m = dict(shared)
        m["xT"] = bf(x[c * TL:(c + 1) * TL].T)
        m["cb4"] = bf(np.concatenate([
            gsel, np.asarray(inputs["be2"], np.float32),
            noise[c * TL:(c + 1) * TL].T], 1))
        crow = np.zeros((NC, 3), np.float32)
        crow[:, 0] = (np.arange(NC) < c)
        crow[0:E, 1] = np.asarray(inputs["b_route"], np.float32)
        crow[0:E, 2] = np.asarray(inputs["b_noise"], np.float32)
        m["crow"] = crow
        in_maps.append(m)
    return in_maps


def _run(inputs, trace=False):
    if "nc" not in _cache:
        _cache["nc"] = _build()
    nc = _cache["nc"]
    in_maps = _prep_inputs(inputs)
    res = run_bass_kernel_spmd(nc, in_maps, core_ids=list(range(NC)),
                               trace=trace)
    _cache["last_res"] = res
    shards = [res.results[c]["out"] for c in range(NC)]   # each [D, TL]
    out = np.concatenate([np.asarray(s, np.float32).T for s in shards],
                         0).reshape(B, N, D)
    return out.astype(np.float32), res.exec_time_ns


def kernel(**inputs):
    out, _ = _run(inputs, trace=False)
    return out


# revision 45
# speedup vs baseline: 1.1239x; 1.0020x over previous
"""Trainium2 Bass kernel for nn_BlockMoEAdapters (8 NeuronCores, SPMD).

Sharding: tokens (B*N = 4096) split contiguously across 8 cores (512 each).
Cores 0-3 hold batch 0, cores 4-7 batch 1. Attention K/V are all-gathered
(fp8, split into two half-collectives issued right after their producing
GEMMs) within each 4-core batch group; MoE capacity ranks use a tiny 8-core
all-gather of per-core expert counts.

Speed notes (measured on HW, not the cost model): fp8 and bf16 matmuls both
run at 1 col/cycle; fp8e4 DoubleRow fuses 2 k-tiles per instruction at the
same throughput (halves instruction count). fp8 is still a big win for DMA
bytes, SBUF footprint, and the k/v collectives. The device fp8e4 is IEEE
e4m3 (bias 8, max 240) - hosts quantize with ml_dtypes.float8_e4m3.
Schedule: k+v are gathered in four quarter-collectives issued as soon as
each quarter's GEMM lands, so attention starts ~50us earlier; all small
constants are packed into 4 DMAs (tiny DMAs cost ~0.6us of queue time
each); the MoE capacity all-gather (32B, latency/skew-bound) is hidden by
splitting the output GEMM into a counts-independent mlp2 phase (accumulated
in-place over xres) and a tiny moe2+be2 phase afterwards; LN2 stats
accumulate inside the proj loop. Weight prescale 8x for the fp8 GEMMs
(dequant folded into exp scale / gelu scale / a proj-epilogue copy); we2 is
quantized unscaled so it can share the moe2 PSUM with bf16-free epilogues.
Softmax denominators ride as a ones-column in V (accumulated by the same
DR matmuls as ao) and are inverted with the fast approx reciprocal;
normalization is self-consistent with the fp8-quantized exp scores so fp8
attention error mostly cancels.
"""
import sys

for _p in ('/opt/trn_rl_repo',):
    if _p not in sys.path:
        sys.path.append(_p)

import ml_dtypes
import numpy as np

import concourse.bass as bass
import concourse.mybir as mybir
import concourse.tile as tile
from concourse import bacc
from concourse.bass_utils import run_bass_kernel_spmd

F32 = mybir.dt.float32
F32R = mybir.dt.float32r
BF16 = mybir.dt.bfloat16
F8 = mybir.dt.float8e4
AF = mybir.ActivationFunctionType
ALU = mybir.AluOpType
DR = mybir.MatmulPerfMode.DoubleRow

B, N, D = 2, 2048, 1024
H, HD = 16, 64
E, TOPK = 4, 2
MOEH, MLPH = 256, 4096
T = B * N
NC = 8
TL = T // NC          # 512 tokens per core
NT = TL // 128        # 4 token tiles
DT = D // 128         # 8 channel tiles
CAP = int(T * TOPK / E * 1.0)   # 2048
GRP = 4               # cores per kv-gather group
EPS = 1e-5
WS = 8.0              # fp8 weight prescale (dequant folded into epilogues)
SEXP = 0.125 / (WS * WS)   # exp scale: hd^-0.5 and two 8x weight scales

_cache = {}


def _mm(nc, out, lhsT, rhs, start, stop, dt=None):
    if dt is not None:
        lhsT, rhs = lhsT.bitcast(dt), rhs.bitcast(dt)
    nc.tensor.matmul(out, lhsT, rhs, start=start, stop=stop)


def _mmdr(nc, out, lhsT, rhs, start, stop):
    nc.tensor.matmul(out, lhsT, rhs, start=start, stop=stop, perf_mode=DR)


def _build():
    nc = bacc.Bacc("TRN2", target_bir_lowering=False, debug=False,
                   num_devices=NC)

    def din(name, shape, dt=F32):
        return nc.dram_tensor(name, list(shape), dt, kind="ExternalInput")

    xT_d = din("xT", (D, TL), BF16)
    # host-retiled weight slabs (see _prep_inputs for layouts)
    wqk_d = din("wqk_l", (128, 16 * DT * 128), F8)
    wv_d = din("wv_l", (128, 2 * DT * 512), F8)
    wproj_d = din("wproj_l", (128, DT * DT * 128), F8)
    wmlp1_d = din("wmlp1_l", (128, 32 * DT * 128), BF16)
    we1_d = din("we1_l", (128, 8 * DT * 128), F8)
    wout8_d = din("wout8_l", (128, DT * 8 * 128), F8)
    wm2_d = din("wm2_l", (128, DT * 32 * 128), BF16)
    cf32_d = din("cf32", (128, 472))         # packed f32 consts
    cbf_d = din("cbf", (128, 128), BF16)     # packed bf16 consts
    cb4_d = din("cb4", (E, 2048), BF16)      # gsel | be2 | noiseT
    crow_d = din("crow", (NC, 3))            # wpfx | broute | bnoise

    out_d = nc.dram_tensor("out", [D, TL], F32, kind="ExternalOutput")

    rg_kv = [[0, 1, 2, 3], [4, 5, 6, 7]]
    rg_all = [list(range(NC))]

    with tile.TileContext(nc) as tc:
        with (
            tc.tile_pool(name="dram", bufs=1, space="DRAM") as dpool,
            tc.tile_pool(name="consts", bufs=1) as cpool,
            tc.tile_pool(name="persist", bufs=1) as ppool,
            tc.tile_pool(name="ps_big", bufs=2, space="PSUM") as ps_big,
            tc.tile_pool(name="ps_bc", bufs=2, space="PSUM") as ps_bc,
            tc.tile_pool(name="ps_ao", bufs=2, space="PSUM") as ps_ao,
            tc.tile_pool(name="wslab", bufs=2) as wpool,
            tc.tile_pool(name="scratch", bufs=2) as spool,
        ):
            # ---------- collective bounce buffers (k+v packed, quarters) --
            KB = 256 * TL                 # k bytes per quarter (2 slabs)
            VB = 128 * 2 * 640            # v bytes per quarter (2 pairs)
            HB = KB + VB
            kv_in = [dpool.tile([1, HB], F8, name=f"kv_in{q_}")
                     for q_ in range(4)]
            kv_out = [dpool.tile([1, GRP * HB], F8, name=f"kv_out{q_}")
                      for q_ in range(4)]

            def kin_k(q):    # [256, TL] view of the k region
                return kv_in[q][0:1, 0:KB].rearrange(
                    "a (p t) -> (a p) t", t=TL)

            def kin_v(q):    # [128, 1280] view of the v region
                return kv_in[q][0:1, KB:HB].rearrange(
                    "a (p c) -> (a p) c", c=2 * 640)

            def kout_k(q, r):
                return kv_out[q][0:1, r * HB:r * HB + KB].rearrange(
                    "a (p t) -> (a p) t", t=TL)

            def kout_v(q, r):
                return kv_out[q][0:1, r * HB + KB:(r + 1) * HB].rearrange(
                    "a (p c) -> (a p) c", c=2 * 640)
            cnt_in = dpool.tile([1, E], F32, name="cnt_in")
            cnt_out = dpool.tile([NC, E], F32, name="cnt_out")

            # ---------- constants (4 packed DMAs on gpsimd) ----------
            cbf = cpool.tile([128, 128], BF16, tag="cbf", name="cbf")
            nc.gpsimd.dma_start(cbf[:], cbf_d[:])
            cf32 = cpool.tile([128, 472], F32, tag="cf32", name="cf32")
            nc.gpsimd.dma_start(cf32[:], cf32_d[:])
            cb4 = cpool.tile([E, 2048], BF16, tag="cb4", name="cb4")
            nc.gpsimd.dma_start(cb4[:], cb4_d[:])
            crow = cpool.tile([NC, 3], F32, tag="crow", name="crow")
            nc.gpsimd.dma_start(crow[:], crow_d[:])
            ln1g_sb = cf32[:, 0:8]
            ln1b_sb = cf32[:, 8:16]
            ln2g_sb = cf32[:, 16:24]
            ln2b_sb = cf32[:, 24:32]
            bproj_sb = cf32[:, 32:40]
            be1_sb = cf32[:, 40:48]
            bmlp1_sb = cf32[:, 48:80]
            bmlp2_sb = cf32[:, 80:88]
            ones_sb = cf32[:, 88:216]
            eye_sb = cf32[:, 216:344]
            utri_sb = cf32[:, 344:472]
            onesb_sb = cbf[:, 0:1]
            c116_sb = cbf[0:1, 0:64]
            wroute_sb = cbf[:, 64:96]
            wnoise_sb = cbf[:, 96:128]
            gsel_sb = cb4[:, 0:512]
            be2_sb = cb4[:, 512:1536]
            noiseT_sb = cb4[:, 1536:2048]
            wpfx_sb = crow[:, 0:1]
            broute_sb = crow[0:E, 1:2]
            bnoise_sb = crow[0:E, 2:3]

            # ---------- load x (CM, bf16; residual source) ----------
            xT_sb = []
            for j in range(DT):
                t = ppool.tile([128, TL], BF16, tag=f"xT{j}", name=f"xT{j}")
                qx = nc.sync if j % 2 == 0 else nc.scalar
                qx.dma_start(t[:], xT_d[j * 128:(j + 1) * 128, :])
                xT_sb.append(t)

            # channel-major fp8 activations as pair-tiles (k-tile pairs
            # adjacent inside each tile for DR; separate tiles keep the
            # dependency tracking fine-grained)
            def pair_tiles(tag):
                ts = [ppool.tile([128, 2 * TL], F8, tag=f"{tag}{u}",
                                 name=f"{tag}{u}") for u in range(DT // 2)]
                return ts, [t[:].rearrange("p (j t) -> p j t", t=TL)
                            for t in ts]

            x1T_t, x1T_p = pair_tiles("x1T")
            aoT_t, aoT_p = pair_tiles("aoT")
            x2T8_t, x2T8_p = pair_tiles("x2T8")
            Hg_t, Hg_p = pair_tiles("Hg")

            class PV:
                def __init__(self, parts):
                    self.parts = parts

                def __getitem__(self, idx):
                    # [:, j, :] or [:, 2u:2u+2, :] or [:, j, a:b]
                    _, js, ts_ = idx
                    if isinstance(js, slice):
                        u = js.start // 2
                        return self.parts[u][:, :, ts_]
                    return self.parts[js // 2][:, js % 2, ts_]

            x1T_v = PV(x1T_p)
            aoT_v = PV(aoT_p)
            x2T8_v = PV(x2T8_p)
            Hg_v = PV(Hg_p)

            # ---------- LayerNorm in CM (bf16 inputs) ----------
            def ln_stats_j(stat, xt, j):
                # accumulate mean/sq sums for channel tile j
                ones_col = onesb_sb[:, 0:1]
                _mm(nc, stat[0:1, :], ones_col, xt[:],
                    j == 0, j == DT - 1)
                sq = spool.tile([128, TL], BF16, tag="lnsq", name="lnsq",
                                bufs=2)
                nc.vector.tensor_tensor(sq[:], xt[:], xt[:], ALU.mult)
                _mm(nc, stat[32:33, :], ones_col, sq[:],
                    j == 0, j == DT - 1)

            def layernorm_cm(xtiles, g_sb, b_sb, wslice, stat=None):
                # wslice(j) -> destination AP for normalized tile j
                musum = stat[0:1, :]
                sqsum = stat[32:33, :]
                mu = spool.tile([1, TL], F32, tag="lnmu", name="lnmu", bufs=1)
                nc.vector.tensor_scalar_mul(mu[:], musum, 1.0 / D)
                msq = spool.tile([1, TL], F32, tag="lnscr", name="lnmsq",
                                 bufs=2)
                nc.vector.tensor_tensor(msq[:], mu[:], mu[:], ALU.mult)
                var = spool.tile([1, TL], F32, tag="lnscr", name="lnvar",
                                 bufs=2)
                nc.vector.scalar_tensor_tensor(var[:], sqsum, 1.0 / D,
                                               msq[:], ALU.mult, ALU.subtract)
                vare = spool.tile([1, TL], F32, tag="lnscr", name="lnvare",
                                  bufs=2)
                nc.vector.tensor_scalar_add(vare[:], var[:], EPS)
                sd = spool.tile([1, TL], F32, tag="lnscr", name="lnsd",
                                bufs=2)
                nc.scalar.activation(sd[:], vare[:], AF.Sqrt)
                rsig = spool.tile([1, TL], F32, tag="lnrsig", name="lnrsig",
                                  bufs=1)
                rscr = spool.tile([1, TL], F32, tag="lnrscr", name="lnrscr",
                                  bufs=1)
                nc.vector.reciprocal_approx_accurate(out=rsig[:], in_=sd[:],
                                                     scratch=rscr[:])
                mub_ps = ps_bc.tile([128, TL], F32, tag="bc", name="mub")
                _mm(nc, mub_ps[:], ones_sb[0:1, :], mu[:], True, True, F32)
                rsb_ps = ps_bc.tile([128, TL], F32, tag="bc", name="rsb")
                _mm(nc, rsb_ps[:], ones_sb[0:1, :], rsig[:], True, True, F32)
                mub = spool.tile([128, TL], F32, tag="mubsb", name="mubsb",
                                 bufs=1)
                nc.vector.tensor_copy(mub[:], mub_ps[:])
                rsb = spool.tile([128, TL], F32, tag="rsbsb", name="rsbsb",
                                 bufs=1)
                nc.vector.tensor_copy(rsb[:], rsb_ps[:])
                for j in range(DT):
                    t1 = spool.tile([128, TL], F32, tag="lnt1", name="lnt1",
                                    bufs=2)
                    nc.vector.tensor_tensor(t1[:], xtiles[j][:], mub[:],
                                            ALU.subtract)
                    t2 = spool.tile([128, TL], F32, tag="lnt2", name="lnt2",
                                    bufs=2)
                    nc.vector.tensor_tensor(t2[:], t1[:], rsb[:], ALU.mult)
                    nc.vector.tensor_scalar(wslice(j), t2[:], g_sb[:, j:j + 1],
                                            b_sb[:, j:j + 1], ALU.mult,
                                            ALU.add)

            qT_sb = [ppool.tile([128, TL], F8, tag=f"qT{m}", name=f"qT{m}")
                     for m in range(DT)]

            with tc.tile_pool(name="st1", bufs=2) as s1pool:
                stat1 = ps_ao.tile([33, TL], F32, tag="ao", name="lnstat1")
                for j in range(DT):
                    ln_stats_j(stat1, xT_sb[j], j)
                layernorm_cm(xT_sb, ln1g_sb, ln1b_sb,
                             lambda j: x1T_p[j // 2][:, j % 2, :],
                             stat=stat1)

                def qk_slab(m):
                    # one 128-out-ch slab of the q/k GEMM (fp8)
                    slab = wpool.tile([128, DT * 128], F8, tag="qkslab",
                                      name="qkslab", bufs=3)
                    nc.sync.dma_start(
                        slab[:], wqk_d[:, m * 1024:(m + 1) * 1024])
                    sv = slab[:].rearrange("p (k c) -> p k c", c=128)
                    ps = ps_big.tile([128, TL], F32, tag="big", name="qk")
                    for u in range(DT // 2):
                        _mmdr(nc, ps[:], sv[:, 2 * u:2 * u + 2, :],
                              x1T_v[:, 2 * u:2 * u + 2, :],
                              u == 0, u == DT // 2 - 1)
                    if m < DT:
                        nc.vector.tensor_copy(qT_sb[m][:], ps[:])
                    else:
                        ksb = s1pool.tile([128, TL], F8, tag="kevac",
                                          name="kevac", bufs=2)
                        nc.vector.tensor_copy(ksb[:], ps[:])
                        mk = m - DT
                        nc.sync.dma_start(
                            kin_k(mk // 2)[(mk % 2) * 128:(mk % 2 + 1) * 128,
                                           :], ksb[:])

                wv_sb = [None, None]

                def v_quarter(qq):
                    # 256 v-channels (2 pairs), TM orientation. pad layout
                    # [q(2), hh(2), mt(4), 80]: col 64 = ones so the ao
                    # matmul also accumulates softmax denominators.
                    nn, iq = qq // 2, qq % 2
                    if iq == 0:
                        wv_sb[nn] = s1pool.tile([128, DT * 512], F8,
                                                tag=f"wv{nn}", name="wv",
                                                bufs=1)
                        nc.gpsimd.dma_start(
                            wv_sb[nn][:],
                            wv_d[:, nn * 4096:(nn + 1) * 4096])
                    wv_v = wv_sb[nn][:].rearrange("p (k c) -> p k c", c=512)
                    vp = s1pool.tile([128, 2 * 640], F8, tag="vpad",
                                     name="vpad", bufs=2)
                    nc.vector.memset(vp[:], 1.0)
                    dst = vp[:].rearrange("p (q hh m c) -> p q hh m c",
                                          hh=2, m=NT, c=80)
                    for mt in range(NT):         # 4 token Mtiles
                        ps = ps_big.tile([128, 256], F32, tag="big",
                                         name="vps")
                        for kk in range(DT):
                            _mm(nc, ps[:],
                                x1T_v[:, kk, mt * 128:(mt + 1) * 128],
                                wv_v[:, kk, iq * 256:(iq + 1) * 256],
                                kk == 0, kk == DT - 1)
                        nc.vector.tensor_copy(
                            dst[:, :, :, mt, 0:64],
                            ps[:].rearrange("p (q hh c) -> p q hh c",
                                            hh=2, c=64))
                    nc.sync.dma_start(kin_v(qq), vp[:])

                def ag(src, dst):
                    nc.gpsimd.collective_compute(
                        "AllGather", ALU.bypass, replica_groups=rg_kv,
                        ins=[src[:].opt()], outs=[dst[:].opt()])

                # per quarter: 2 k slabs + v quarter -> packed AG; then q
                for qq in range(4):
                    qk_slab(DT + 2 * qq)
                    qk_slab(DT + 2 * qq + 1)
                    v_quarter(qq)
                    ag(kv_in[qq], kv_out[qq])
                for m in range(DT):
                    qk_slab(m)

            # ---------- preload all MoE first-layer weights (1 DMA) -------
            we1_all = cpool.tile([128, 8 * DT * 128], F8, tag="we1a",
                                 name="we1a")
            nc.gpsimd.dma_start(we1_all[:], we1_d[:])
            we1_v = we1_all[:].rearrange("p (m k c) -> p m k c", k=DT, c=128)

            # ---------- attention (2-head interleave, fp8, DR ao) ----------
            with (
                tc.tile_pool(name="attn", bufs=2) as apool,
                tc.tile_pool(name="vsb", bufs=2) as vpool,
                tc.tile_pool(name="ssb", bufs=4) as spool_s,
            ):
                for p in range(DT):              # head pair
                    hf, pq = p // 2, p % 2       # kv quarter, pair in qtr
                    kp = []
                    vt = []
                    for r in range(GRP):
                        kt_ = apool.tile([128, TL], F8, tag=f"kp{r}",
                                         name=f"kp{r}")
                        nc.sync.dma_start(
                            kt_[:],
                            kout_k(hf, r)[pq * 128:(pq + 1) * 128, :])
                        kp.append(kt_)
                        vt_ = vpool.tile([128, 640], F8, tag=f"vt{r}",
                                         name=f"vt{r}")
                        nc.sync.dma_start(
                            vt_[:],
                            kout_v(hf, r)[:, pq * 640:(pq + 1) * 640])
                        vt.append(vt_)
                    ao_ps = [ps_ao.tile([66, TL], F32, tag="ao",
                                        name=f"ao{hh}") for hh in range(2)]
                    for beat in range(8):        # 2 key tiles per beat
                        for hh in range(2):
                            po = 64 * hh
                            s_sb = spool_s.tile([128, 2 * TL], F8,
                                                tag="ssb", name="ssb")
                            s2 = ps_big.tile([128, 2 * TL], F32, tag="big",
                                             name="s2")
                            for u in range(2):
                                kt = 2 * beat + u
                                r, cc = kt // 4, kt % 4
                                _mm(nc, s2[:, u * TL:(u + 1) * TL],
                                    kp[r][po:po + 64,
                                          cc * 128:(cc + 1) * 128],
                                    qT_sb[p][po:po + 64, :], True, True)
                            nc.scalar.activation(s_sb[:], s2[:],
                                                 AF.Exp, scale=SEXP)
                            r, cc = (2 * beat) // 4, (2 * beat) % 4
                            vtv = vt[r][:].rearrange(
                                "p (hh m c) -> p hh m c", hh=2, c=80)
                            _mmdr(nc, ao_ps[hh][:],
                                  vtv[:, hh, cc:cc + 2, 0:66],
                                  s_sb[:].rearrange("p (u t) -> p u t",
                                                    t=TL),
                                  beat == 0, beat == 7)
                    for hh in range(2):
                        po = 64 * hh
                        dsb = spool_s.tile([1, TL], F32, tag="densb",
                                           name="densb", bufs=1)
                        nc.vector.tensor_copy(dsb[:], ao_ps[hh][64:65, :])
                        recip = spool_s.tile([1, TL], F32, tag="recip",
                                             name="recip", bufs=1)
                        nc.vector.reciprocal_approx_fast(
                            out=recip[:], in_=dsb[:])
                        recb = spool_s.tile([1, TL], BF16, tag="recb",
                                            name="recb", bufs=1)
                        nc.vector.tensor_copy(recb[:], recip[:])
                        bc_ps = ps_bc.tile([64, TL], F32, tag="bc",
                                           name="aobc")
                        _mm(nc, bc_ps[:], c116_sb, recb[:],
                            True, True)
                        bc_sb = spool_s.tile([64, TL], BF16, tag="aobcsb",
                                             name="aobcsb", bufs=2)
                        nc.vector.tensor_copy(bc_sb[:], bc_ps[:])
                        nc.vector.tensor_tensor(
                            aoT_p[p // 2][po:po + 64, p % 2, :],
                            ao_ps[hh][0:64, :], bc_sb[:], ALU.mult)

                # ---------- proj + residual ----------
                stat2 = ps_ao.tile([33, TL], F32, tag="ao", name="lnstat2")
                xres = []
                for m in range(DT):
                    slab = wpool.tile([128, DT * 128], F8, tag="qkslab",
                                      name="projslab", bufs=3)
                    nc.sync.dma_start(
                        slab[:], wproj_d[:, m * 1024:(m + 1) * 1024])
                    sv = slab[:].rearrange("p (k c) -> p k c", c=128)
                    ps = ps_big.tile([128, TL], F32, tag="big", name="proj")
                    for u in range(DT // 2):
                        _mmdr(nc, ps[:], sv[:, 2 * u:2 * u + 2, :],
                              aoT_v[:, 2 * u:2 * u + 2, :],
                              u == 0, u == DT // 2 - 1)
                    pd = spool.tile([128, TL], F32, tag="projdq",
                                    name="projdq", bufs=2)
                    nc.scalar.activation(pd[:], ps[:], AF.Copy,
                                         scale=1.0 / (WS * WS))
                    xr = ppool.tile([128, TL], BF16, tag=f"xres{m}",
                                    name=f"xres{m}")
                    nc.vector.scalar_tensor_tensor(
                        xr[:], pd[:], bproj_sb[:, m:m + 1], xT_sb[m][:],
                        ALU.add, ALU.add)
                    xres.append(xr)
                    ln_stats_j(stat2, xr, m)

            # ---------- LN2 (bf16 out + fp8 copy) ----------
            x2T = [ppool.tile([128, TL], BF16, tag=f"x2T{j}", name=f"x2T{j}")
                   for j in range(DT)]
            layernorm_cm(xres, ln2g_sb, ln2b_sb,
                         lambda j: x2T[j][:], stat=stat2)

            # ---------- router ----------
            rt_ps = ps_ao.tile([32 + E, TL], F32, tag="ao", name="rt")
            logit_ps = rt_ps[0:E, :]
            nlin_ps = rt_ps[32:32 + E, :]
            for j in range(DT):
                _mm(nc, logit_ps, wroute_sb[:, j * E:(j + 1) * E],
                    x2T[j][:], j == 0, j == DT - 1)
            for j in range(DT):
                _mm(nc, nlin_ps, wnoise_sb[:, j * E:(j + 1) * E],
                    x2T[j][:], j == 0, j == DT - 1)
            logits = spool.tile([E, TL], F32, tag="logits", name="logits",
                                bufs=1)
            nc.vector.tensor_scalar(logits[:], logit_ps,
                                    broute_sb[:, 0:1], None, ALU.add)
            spe = spool.tile([E, TL], BF16, tag="softpe", name="softpe",
                             bufs=1)
            nc.scalar.activation(spe[:], nlin_ps, AF.Exp,
                                 bias=bnoise_sb[:, 0:1])
            spe1 = spool.tile([E, TL], BF16, tag="softpe1", name="softpe1",
                              bufs=1)
            nc.vector.tensor_scalar_add(spe1[:], spe[:], 1.0)
            sp = spool.tile([E, TL], BF16, tag="softp", name="softp",
                            bufs=1)
            nc.scalar.activation(sp[:], spe1[:], AF.Ln)
            nsp = spool.tile([E, TL], BF16, tag="nsp", name="nsp", bufs=1)
            nc.vector.tensor_tensor(nsp[:], noiseT_sb, sp[:], ALU.mult)
            noisy_cm = spool.tile([E, TL], F32, tag="noisycm", name="noisycm",
                                  bufs=1)
            nc.vector.tensor_tensor(noisy_cm[:], nsp[:], logits[:], ALU.add)

            # ---------- top-2 gates (TM) ----------
            noisy8 = ppool.tile([128, 8 * NT], F32, tag="noisy8",
                                name="noisy8")
            nc.vector.memset(noisy8[:], -1e30)
            m8 = ppool.tile([128, 8 * NT], F32, tag="m8", name="m8")
            gate = ppool.tile([128, E * NT], F32, tag="gate", name="gate")
            mask = ppool.tile([128, E * NT], F32, tag="mask", name="mask")
            geT = ppool.tile([E, TL], BF16, tag="geT", name="geT")
            cnt_sb = ppool.tile([1, NT * E], F32, tag="cntsb", name="cntsb")
            for j in range(NT):
                tr_ps = ps_bc.tile([128, E], F32, tag="bc", name="ntr")
                nc.tensor.matmul(tr_ps[:],
                                 noisy_cm[:, j * 128:(j + 1) * 128],
                                 eye_sb[0:E, 0:E], is_transpose=True,
                                 start=True, stop=True)
                nc.vector.tensor_copy(noisy8[:, 8 * j:8 * j + E], tr_ps[:])
            dv = spool.tile([128, NT], F32, tag="dv", name="dv")
            for j in range(NT):
                nc.vector.max(m8[:, 8 * j:8 * j + 8],
                              noisy8[:, 8 * j:8 * j + 8])
                nc.vector.tensor_tensor(dv[:, j:j + 1],
                                        m8[:, 8 * j + 1:8 * j + 2],
                                        m8[:, 8 * j:8 * j + 1],
                                        ALU.subtract)
            p2a = spool.tile([128, NT], F32, tag="p2a", name="p2a")
            nc.scalar.activation(p2a[:], dv[:], AF.Exp)
            dden = spool.tile([128, NT], F32, tag="dden", name="dden")
            nc.vector.tensor_scalar_add(dden[:], p2a[:], 1.0)
            rda = spool.tile([128, NT], F32, tag="rda", name="rda")
            nc.vector.reciprocal(rda[:], dden[:])
            for j in range(NT):
                nm = noisy8[:, 8 * j:8 * j + E]
                v1 = m8[:, 8 * j:8 * j + 1]
                v2 = m8[:, 8 * j + 1:8 * j + 2]
                oh1 = spool.tile([128, E], F32, tag="oh1", name="oh1")
                nc.vector.tensor_scalar(oh1[:], nm, v1, None, ALU.is_ge)
                msk = mask[:, E * j:E * (j + 1)]
                nc.vector.tensor_scalar(msk, nm, v2, None, ALU.is_ge)
                oh2 = spool.tile([128, E], F32, tag="oh2", name="oh2")
                nc.vector.tensor_tensor(oh2[:], msk, oh1[:], ALU.subtract)
                gnum = spool.tile([128, E], F32, tag="gnum", name="gnum")
                nc.vector.tensor_scalar(gnum[:], oh2[:], p2a[:, j:j + 1],
                                        None, ALU.mult)
                gnum2 = spool.tile([128, E], F32, tag="gnum2", name="gnum2")
                nc.vector.tensor_tensor(gnum2[:], gnum[:], oh1[:], ALU.add)
                nc.vector.tensor_scalar(gate[:, E * j:E * (j + 1)],
                                        gnum2[:], rda[:, j:j + 1], None,
                                        ALU.mult)
                cps = ps_bc.tile([1, E], F32, tag="bc", name="cnt")
                _mm(nc, cps[:], ones_sb[:, 0:1], msk, True, True, F32)
                nc.vector.tensor_copy(cnt_sb[0:1, E * j:E * (j + 1)], cps[:])

            # total counts -> all-gather
            tot = spool.tile([1, E], F32, tag="cnttot", name="cnttot",
                             bufs=1)
            nc.vector.tensor_tensor(tot[:], cnt_sb[0:1, 0:E],
                                    cnt_sb[0:1, E:2 * E], ALU.add)
            nc.vector.tensor_tensor(tot[:], tot[:], cnt_sb[0:1, 2 * E:3 * E],
                                    ALU.add)
            nc.vector.tensor_tensor(tot[:], tot[:], cnt_sb[0:1, 3 * E:4 * E],
                                    ALU.add)
            nc.sync.dma_start(cnt_in[:], tot[:])

            # ---------- MLP hidden + MoE hidden (overlaps counts AG) ------
            for j in range(DT):
                nc.scalar.activation(x2T8_p[j // 2][:, j % 2, :],
                                     x2T[j][:], AF.Copy)
            Hmoe = []
            for me in range(2 * E):
                ps = ps_big.tile([128, TL], F32, tag="big", name="hmoe")
                for u in range(DT // 2):
                    _mmdr(nc, ps[:], we1_v[:, me, 2 * u:2 * u + 2, :],
                          x2T8_v[:, 2 * u:2 * u + 2, :],
                          u == 0, u == DT // 2 - 1)
                hs = ppool.tile([128, TL], BF16, tag=f"hmoe{me}",
                                name=f"hmoe{me}")
                nc.scalar.activation(
                    hs[:], ps[:], AF.Gelu, scale=1.0 / WS,
                    bias=be1_sb[:, me:me + 1])
                Hmoe.append(hs)

            Hm_sb = []
            for m in range(MLPH // 128):
                slab = wpool.tile([128, DT * 128], BF16, tag="m1slab",
                                  name="m1slab", bufs=6)
                q1 = nc.sync if m % 2 == 0 else nc.gpsimd
                q1.dma_start(
                    slab[:], wmlp1_d[:, m * 1024:(m + 1) * 1024])
                ps = ps_big.tile([128, TL], F32, tag="big", name="hm")
                for kk in range(DT):
                    _mm(nc, ps[:], slab[:, kk * 128:(kk + 1) * 128],
                        x2T[kk][:], kk == 0, kk == DT - 1)
                hm = ppool.tile([128, TL], BF16, tag=f"hm{m}", name=f"hm{m}")
                nc.scalar.activation(hm[:], ps[:], AF.Gelu,
                                     bias=bmlp1_sb[:, m:m + 1])
                Hm_sb.append(hm)
            nc.gpsimd.collective_compute(
                "AllGather", ALU.bypass, replica_groups=rg_all,
                ins=[cnt_in[:].opt()], outs=[cnt_out[:].opt()])

            # ---------- preout: mlp2 GEMM + bias + residual (cnt-free) --
            preout = []
            for m in range(DT):
                slab2 = wpool.tile([128, 32 * 128], BF16, tag="outslab",
                                   name="outslab")
                nc.sync.dma_start(
                    slab2[:], wm2_d[:, m * 4096:(m + 1) * 4096])
                ps = ps_big.tile([128, TL], F32, tag="big", name="pre")
                for kk in range(MLPH // 128):
                    _mm(nc, ps[:], slab2[:, kk * 128:(kk + 1) * 128],
                        Hm_sb[kk][:], kk == 0, kk == MLPH // 128 - 1)
                nc.vector.scalar_tensor_tensor(
                    xres[m][:], ps[:], bmlp2_sb[:, m:m + 1], xres[m][:],
                    ALU.add, ALU.add)
                preout.append(xres[m])

            # ---------- ranks / keep / gate_eff ----------
            cntg = spool.tile([NC, E], F32, tag="cntg", name="cntg", bufs=1)
            nc.sync.dma_start(cntg[:], cnt_out[:])
            off_ps = ps_bc.tile([1, E], F32, tag="bc", name="off")
            _mm(nc, off_ps[:], wpfx_sb, cntg[:], True, True, F32)
            car = spool.tile([1, E * NT], F32, tag="car", name="car", bufs=1)
            nc.vector.tensor_copy(car[:, 0:E], off_ps[:])
            for j in range(1, NT):
                nc.vector.tensor_tensor(car[:, E * j:E * (j + 1)],
                                        car[:, E * (j - 1):E * j],
                                        cnt_sb[0:1, E * (j - 1):E * j],
                                        ALU.add)
            ge_tm = ppool.tile([128, E * NT], F32, tag="getm", name="getm")
            for j in range(NT):
                rk_ps = ps_bc.tile([128, E], F32, tag="bc", name="rank")
                _mm(nc, rk_ps[:], utri_sb,
                    mask[:, E * j:E * (j + 1)], True, False, F32)
                _mm(nc, rk_ps[:], ones_sb[0:1, :],
                    car[:, E * j:E * (j + 1)], False, True, F32)
                keep = spool.tile([128, E], F32, tag="keep", name="keep")
                nc.vector.tensor_scalar(keep[:], rk_ps[:], float(CAP), None,
                                        ALU.is_lt)
                nc.vector.tensor_tensor(ge_tm[:, E * j:E * (j + 1)],
                                        gate[:, E * j:E * (j + 1)],
                                        keep[:], ALU.mult)
            for j in range(NT):
                tr_ps = ps_bc.tile([E, 128], F32, tag="bc", name="getr")
                nc.tensor.matmul(tr_ps[:], ge_tm[:, E * j:E * (j + 1)],
                                 eye_sb, is_transpose=True,
                                 start=True, stop=True)
                nc.vector.tensor_copy(geT[:, j * 128:(j + 1) * 128], tr_ps[:])

            # gate the MoE hidden
            for e in range(E):
                bc_ps = ps_bc.tile([128, TL], F32, tag="bc", name="gbc")
                _mm(nc, bc_ps[:], gsel_sb[:, e * 128:(e + 1) * 128],
                    geT[:], True, True)
                bc_sb = spool.tile([128, TL], BF16, tag="gbcsb", name="gbcsb",
                                   bufs=2)
                nc.vector.tensor_copy(bc_sb[:], bc_ps[:])
                for hmi in range(MOEH // 128):
                    me = 2 * e + hmi
                    nc.vector.tensor_tensor(Hg_p[me // 2][:, me % 2, :],
                                            Hmoe[me][:],
                                            bc_sb[:], ALU.mult)

            # ---------- output GEMM: moe(DR fp8) + be2, add preout --------
            for m in range(DT):
                slab8 = wpool.tile([128, 8 * 128], F8, tag="out8",
                                   name="out8")
                nc.gpsimd.dma_start(
                    slab8[:], wout8_d[:, m * 1024:(m + 1) * 1024])
                sv8 = slab8[:].rearrange("p (k c) -> p k c", c=128)
                ps = ps_big.tile([128, TL], F32, tag="big", name="out")
                for u in range(4):           # we2 DR pairs
                    _mmdr(nc, ps[:], sv8[:, 2 * u:2 * u + 2, :],
                          Hg_v[:, 2 * u:2 * u + 2, :], u == 0, False)
                _mm(nc, ps[:], be2_sb[:, m * 128:(m + 1) * 128],
                    geT[:], False, True)
                o = spool.tile([128, TL], F32, tag="outsb", name="outsb",
                               bufs=2)
                nc.vector.tensor_tensor(o[:], ps[:], preout[m][:], ALU.add)
                nc.sync.dma_start(out_d[m * 128:(m + 1) * 128, :], o[:])

    nc.compile()
    return nc


def _tile_lhst(w, n_k, n_m):
    # w: [n_k*128, n_m*128] -> [128, n_m, n_k, 128] -> [128, n_m*n_k*128]
    kdim, mdim = w.shape
    return np.ascontiguousarray(
        w.reshape(n_k, 128, n_m, 128).transpose(1, 2, 0, 3)
        .reshape(128, n_m * n_k * 128))


def _prep_inputs(inputs):
    f32 = lambda a: np.ascontiguousarray(np.asarray(a, np.float32))
    bf = lambda a: np.ascontiguousarray(
        np.asarray(a, np.float32).astype(ml_dtypes.bfloat16))
    f8 = lambda a, s=1.0: np.ascontiguousarray(
        (np.asarray(a, np.float32) * s).astype(ml_dtypes.float8_e4m3))
    x = f32(inputs["x"]).reshape(T, D)
    noise = f32(inputs["noise"]).reshape(T, E)
    w_qkv = np.asarray(inputs["w_qkv"], np.float32)
    wqkT = w_qkv[:2 * D].T                       # [D, 2048]
    wvT = w_qkv[2 * D:].T                        # [D, D]
    wprojT = np.asarray(inputs["w_proj"], np.float32).T
    we1 = np.asarray(inputs["we1"], np.float32)  # [E, D, MOEH]
    we2 = np.asarray(inputs["we2"], np.float32)  # [E, MOEH, D]
    wmlp1 = np.asarray(inputs["w_mlp1"], np.float32)   # [D, MLPH]
    wmlp2 = np.asarray(inputs["w_mlp2"], np.float32)   # [MLPH, D]

    # we1 slabs: m-index = e*2+hmi over [D, 256] each
    we1_flat = np.concatenate([we1[e] for e in range(E)], 1)  # [D, E*MOEH]
    # wout8: per m, 8 we2 tiles (e,hmi); wm2: per m, 32 wmlp2 tiles
    we2_l = we2.reshape(E, 2, 128, DT, 128).transpose(2, 3, 0, 1, 4) \
        .reshape(128, DT * 8 * 128)
    wm2_l = wmlp2.reshape(32, 128, DT, 128).transpose(1, 2, 0, 3) \
        .reshape(128, DT * 32 * 128)
    # wv: [128, nn, kk, 512]
    wv_l = wvT.reshape(DT, 128, 2, 512).transpose(1, 2, 0, 3) \
        .reshape(128, 2 * DT * 512)

    cols = lambda a, n: np.asarray(a, np.float32).reshape(n, 128).T
    cf32 = np.concatenate([
        cols(inputs["ln1_g"], DT), cols(inputs["ln1_b"], DT),
        cols(inputs["ln2_g"], DT), cols(inputs["ln2_b"], DT),
        cols(inputs["b_proj"], DT), cols(inputs["be1"], DT),
        cols(inputs["b_mlp1"], 32), cols(inputs["b_mlp2"], DT),
        np.ones((128, 128), np.float32),
        np.eye(128, dtype=np.float32),
        np.triu(np.ones((128, 128), np.float32), 1),
    ], 1)
    cbf = np.concatenate([
        np.ones((128, 64), np.float32),
        np.asarray(inputs["w_route"], np.float32).reshape(DT, 128, E)
        .transpose(1, 0, 2).reshape(128, DT * E),
        np.asarray(inputs["w_noise"], np.float32).reshape(DT, 128, E)
        .transpose(1, 0, 2).reshape(128, DT * E),
    ], 1)
    gsel = np.repeat(np.eye(E, dtype=np.float32), 128, 1)

    shared = dict(
        wqk_l=f8(_tile_lhst(wqkT, DT, 16), WS),
        wv_l=f8(wv_l, WS),
        wproj_l=f8(_tile_lhst(wprojT, DT, DT), WS),
        wmlp1_l=bf(_tile_lhst(wmlp1, DT, 32)),
        we1_l=f8(_tile_lhst(we1_flat, DT, 8), WS),
        wout8_l=f8(we2_l),
        wm2_l=bf(wm2_l),
        cf32=f32(cf32),
        cbf=bf(cbf),
    )
    in_maps = []
    for c in range(NC):
        m = dict(shared)
        m["xT"] = bf(x[c * TL:(c + 1) * TL].T)
        m["cb4"] = bf(np.concatenate([
            gsel, np.asarray(inputs["be2"], np.float32),
            noise[c * TL:(c + 1) * TL].T], 1))
        crow = np.zeros((NC, 3), np.float32)
        crow[:, 0] = (np.arange(NC) < c)
        crow[0:E, 1] = np.asarray(inputs["b_route"], np.float32)
        crow[0:E, 2] = np.asarray(inputs["b_noise"], np.float32)
        m["crow"] = crow
        in_maps.append(m)
    return in_maps


def _run(inputs, trace=False):
    if "nc" not in _cache:
        _cache["nc"] = _build()
    nc = _cache["nc"]
    in_maps = _prep_inputs(inputs)
    res = run_bass_kernel_spmd(nc, in_maps, core_ids=list(range(NC)),
                               trace=trace)
    _cache["last_res"] = res
    shards = [res.results[c]["out"] for c in range(NC)]   # each [D, TL]
    out = np.concatenate([np.asarray(s, np.float32).T for s in shards],
                         0).reshape(B, N, D)
    return out.astype(np.float32), res.exec_time_ns


def kernel(**inputs):
    out, _ = _run(inputs, trace=False)
    return out


# revision 46
# speedup vs baseline: 1.1316x; 1.0068x over previous
"""Trainium2 Bass kernel for nn_BlockMoEAdapters (8 NeuronCores, SPMD).

Sharding: tokens (B*N = 4096) split contiguously across 8 cores (512 each).
Cores 0-3 hold batch 0, cores 4-7 batch 1. Attention K/V are all-gathered
(fp8, split into two half-collectives issued right after their producing
GEMMs) within each 4-core batch group; MoE capacity ranks use a tiny 8-core
all-gather of per-core expert counts.

Speed notes (measured on HW, not the cost model): fp8 and bf16 matmuls both
run at 1 col/cycle; fp8e4 DoubleRow fuses 2 k-tiles per instruction at the
same throughput (halves instruction count). fp8 is still a big win for DMA
bytes, SBUF footprint, and the k/v collectives. The device fp8e4 is IEEE
e4m3 (bias 8, max 240) - hosts quantize with ml_dtypes.float8_e4m3.
Schedule: k+v are gathered in four quarter-collectives issued as soon as
each quarter's GEMM lands, so attention starts ~50us earlier; all small
constants are packed into 4 DMAs (tiny DMAs cost ~0.6us of queue time
each); the MoE capacity all-gather (32B, latency/skew-bound) is hidden by
splitting the output GEMM into a counts-independent mlp2 phase (accumulated
in-place over xres) and a tiny moe2+be2 phase afterwards; LN2 stats
accumulate inside the proj loop. Weight prescale 8x for the fp8 GEMMs
(dequant folded into exp scale / gelu scale / a proj-epilogue copy); we2 is
quantized unscaled so it can share the moe2 PSUM with bf16-free epilogues.
Softmax denominators ride as a ones-column in V (accumulated by the same
DR matmuls as ao) and are inverted with the fast approx reciprocal;
normalization is self-consistent with the fp8-quantized exp scores so fp8
attention error mostly cancels.
"""
import sys

for _p in ('/opt/trn_rl_repo',):
    if _p not in sys.path:
        sys.path.append(_p)

import ml_dtypes
import numpy as np

import concourse.bass as bass
import concourse.mybir as mybir
import concourse.tile as tile
from concourse import bacc
from concourse.bass_utils import run_bass_kernel_spmd

F32 = mybir.dt.float32
F32R = mybir.dt.float32r
BF16 = mybir.dt.bfloat16
F8 = mybir.dt.float8e4
AF = mybir.ActivationFunctionType
ALU = mybir.AluOpType
DR = mybir.MatmulPerfMode.DoubleRow

B, N, D = 2, 2048, 1024
H, HD = 16, 64
E, TOPK = 4, 2
MOEH, MLPH = 256, 4096
T = B * N
NC = 8
TL = T // NC          # 512 tokens per core
NT = TL // 128        # 4 token tiles
DT = D // 128         # 8 channel tiles
CAP = int(T * TOPK / E * 1.0)   # 2048
GRP = 4               # cores per kv-gather group
EPS = 1e-5
WS = 8.0              # fp8 weight prescale (dequant folded into epilogues)
SEXP = 0.125 / (WS * WS)   # exp scale: hd^-0.5 and two 8x weight scales

_cache = {}


def _mm(nc, out, lhsT, rhs, start, stop, dt=None):
    if dt is not None:
        lhsT, rhs = lhsT.bitcast(dt), rhs.bitcast(dt)
    nc.tensor.matmul(out, lhsT, rhs, start=start, stop=stop)


def _mmdr(nc, out, lhsT, rhs, start, stop):
    nc.tensor.matmul(out, lhsT, rhs, start=start, stop=stop, perf_mode=DR)


def _build():
    nc = bacc.Bacc("TRN2", target_bir_lowering=False, debug=False,
                   num_devices=NC)

    def din(name, shape, dt=F32):
        return nc.dram_tensor(name, list(shape), dt, kind="ExternalInput")

    xT_d = din("xT", (D, TL), BF16)
    # host-retiled weight slabs (see _prep_inputs for layouts)
    wqk_d = din("wqk_l", (128, 16 * DT * 128), F8)
    wv_d = din("wv_l", (128, 2 * DT * 512), F8)
    wproj_d = din("wproj_l", (128, DT * DT * 128), F8)
    wmlp1_d = din("wmlp1_l", (128, 32 * DT * 128), BF16)
    we1_d = din("we1_l", (128, 8 * DT * 128), F8)
    wout8_d = din("wout8_l", (128, DT * 8 * 128), F8)
    wm2_d = din("wm2_l", (128, DT * 32 * 128), BF16)
    cf32_d = din("cf32", (128, 472))         # packed f32 consts
    cbf_d = din("cbf", (128, 128), BF16)     # packed bf16 consts
    cb4_d = din("cb4", (E, 2048), BF16)      # gsel | be2 | noiseT
    crow_d = din("crow", (NC, 3))            # wpfx | broute | bnoise

    out_d = nc.dram_tensor("out", [D, TL], F32, kind="ExternalOutput")

    rg_kv = [[0, 1, 2, 3], [4, 5, 6, 7]]
    rg_all = [list(range(NC))]

    with tile.TileContext(nc) as tc:
        with (
            tc.tile_pool(name="dram", bufs=1, space="DRAM") as dpool,
            tc.tile_pool(name="consts", bufs=1) as cpool,
            tc.tile_pool(name="persist", bufs=1) as ppool,
            tc.tile_pool(name="ps_big", bufs=2, space="PSUM") as ps_big,
            tc.tile_pool(name="ps_bc", bufs=2, space="PSUM") as ps_bc,
            tc.tile_pool(name="ps_ao", bufs=2, space="PSUM") as ps_ao,
            tc.tile_pool(name="wslab", bufs=2) as wpool,
            tc.tile_pool(name="scratch", bufs=2) as spool,
        ):
            # ---------- collective bounce buffers (k+v packed, quarters) --
            KB = 256 * TL                 # k bytes per quarter (2 slabs)
            VB = 128 * 2 * 640            # v bytes per quarter (2 pairs)
            HB = KB + VB
            kv_in = [dpool.tile([1, HB], F8, name=f"kv_in{q_}")
                     for q_ in range(4)]
            kv_out = [dpool.tile([1, GRP * HB], F8, name=f"kv_out{q_}")
                      for q_ in range(4)]

            def kin_k(q):    # [256, TL] view of the k region
                return kv_in[q][0:1, 0:KB].rearrange(
                    "a (p t) -> (a p) t", t=TL)

            def kin_v(q):    # [128, 1280] view of the v region
                return kv_in[q][0:1, KB:HB].rearrange(
                    "a (p c) -> (a p) c", c=2 * 640)

            def kout_k(q, r):
                return kv_out[q][0:1, r * HB:r * HB + KB].rearrange(
                    "a (p t) -> (a p) t", t=TL)

            def kout_v(q, r):
                return kv_out[q][0:1, r * HB + KB:(r + 1) * HB].rearrange(
                    "a (p c) -> (a p) c", c=2 * 640)
            cnt_in = dpool.tile([1, E], F32, name="cnt_in")
            cnt_out = dpool.tile([NC, E], F32, name="cnt_out")

            # ---------- constants (4 packed DMAs on gpsimd) ----------
            cbf = cpool.tile([128, 128], BF16, tag="cbf", name="cbf")
            nc.gpsimd.dma_start(cbf[:], cbf_d[:])
            cf32 = cpool.tile([128, 472], F32, tag="cf32", name="cf32")
            nc.gpsimd.dma_start(cf32[:], cf32_d[:])
            cb4 = cpool.tile([E, 2048], BF16, tag="cb4", name="cb4")
            nc.gpsimd.dma_start(cb4[:], cb4_d[:])
            crow = cpool.tile([NC, 3], F32, tag="crow", name="crow")
            nc.gpsimd.dma_start(crow[:], crow_d[:])
            ln1g_sb = cf32[:, 0:8]
            ln1b_sb = cf32[:, 8:16]
            ln2g_sb = cf32[:, 16:24]
            ln2b_sb = cf32[:, 24:32]
            bproj_sb = cf32[:, 32:40]
            be1_sb = cf32[:, 40:48]
            bmlp1_sb = cf32[:, 48:80]
            bmlp2_sb = cf32[:, 80:88]
            ones_sb = cf32[:, 88:216]
            eye_sb = cf32[:, 216:344]
            utri_sb = cf32[:, 344:472]
            onesb_sb = cbf[:, 0:1]
            c116_sb = cbf[0:1, 0:64]
            wroute_sb = cbf[:, 64:96]
            wnoise_sb = cbf[:, 96:128]
            gsel_sb = cb4[:, 0:512]
            be2_sb = cb4[:, 512:1536]
            noiseT_sb = cb4[:, 1536:2048]
            wpfx_sb = crow[:, 0:1]
            broute_sb = crow[0:E, 1:2]
            bnoise_sb = crow[0:E, 2:3]

            # ---------- load x (CM, bf16; residual source) ----------
            xT_sb = []
            for j in range(DT):
                t = ppool.tile([128, TL], BF16, tag=f"xT{j}", name=f"xT{j}")
                qx = nc.sync if j % 2 == 0 else nc.scalar
                qx.dma_start(t[:], xT_d[j * 128:(j + 1) * 128, :])
                xT_sb.append(t)

            # channel-major fp8 activations as pair-tiles (k-tile pairs
            # adjacent inside each tile for DR; separate tiles keep the
            # dependency tracking fine-grained)
            def pair_tiles(tag):
                ts = [ppool.tile([128, 2 * TL], F8, tag=f"{tag}{u}",
                                 name=f"{tag}{u}") for u in range(DT // 2)]
                return ts, [t[:].rearrange("p (j t) -> p j t", t=TL)
                            for t in ts]

            x1T_t, x1T_p = pair_tiles("x1T")
            aoT_t, aoT_p = pair_tiles("aoT")
            x2T8_t, x2T8_p = pair_tiles("x2T8")
            Hg_t, Hg_p = pair_tiles("Hg")

            class PV:
                def __init__(self, parts):
                    self.parts = parts

                def __getitem__(self, idx):
                    # [:, j, :] or [:, 2u:2u+2, :] or [:, j, a:b]
                    _, js, ts_ = idx
                    if isinstance(js, slice):
                        u = js.start // 2
                        return self.parts[u][:, :, ts_]
                    return self.parts[js // 2][:, js % 2, ts_]

            x1T_v = PV(x1T_p)
            aoT_v = PV(aoT_p)
            x2T8_v = PV(x2T8_p)
            Hg_v = PV(Hg_p)

            # ---------- LayerNorm in CM (bf16 inputs) ----------
            def ln_stats_j(stat, xt, j):
                # accumulate mean/sq sums for channel tile j
                ones_col = onesb_sb[:, 0:1]
                _mm(nc, stat[0:1, :], ones_col, xt[:],
                    j == 0, j == DT - 1)
                sq = spool.tile([128, TL], BF16, tag="lnsq", name="lnsq",
                                bufs=2)
                nc.vector.tensor_tensor(sq[:], xt[:], xt[:], ALU.mult)
                _mm(nc, stat[32:33, :], ones_col, sq[:],
                    j == 0, j == DT - 1)

            def layernorm_cm(xtiles, g_sb, b_sb, wslice, stat=None):
                # wslice(j) -> destination AP for normalized tile j
                musum = stat[0:1, :]
                sqsum = stat[32:33, :]
                mu = spool.tile([1, TL], F32, tag="lnmu", name="lnmu", bufs=1)
                nc.vector.tensor_scalar_mul(mu[:], musum, 1.0 / D)
                msq = spool.tile([1, TL], F32, tag="lnscr", name="lnmsq",
                                 bufs=2)
                nc.vector.tensor_tensor(msq[:], mu[:], mu[:], ALU.mult)
                var = spool.tile([1, TL], F32, tag="lnscr", name="lnvar",
                                 bufs=2)
                nc.vector.scalar_tensor_tensor(var[:], sqsum, 1.0 / D,
                                               msq[:], ALU.mult, ALU.subtract)
                vare = spool.tile([1, TL], F32, tag="lnscr", name="lnvare",
                                  bufs=2)
                nc.vector.tensor_scalar_add(vare[:], var[:], EPS)
                sd = spool.tile([1, TL], F32, tag="lnscr", name="lnsd",
                                bufs=2)
                nc.scalar.activation(sd[:], vare[:], AF.Sqrt)
                rsig = spool.tile([1, TL], F32, tag="lnrsig", name="lnrsig",
                                  bufs=1)
                rscr = spool.tile([1, TL], F32, tag="lnrscr", name="lnrscr",
                                  bufs=1)
                nc.vector.reciprocal_approx_accurate(out=rsig[:], in_=sd[:],
                                                     scratch=rscr[:])
                mub_ps = ps_bc.tile([128, TL], F32, tag="bc", name="mub")
                _mm(nc, mub_ps[:], ones_sb[0:1, :], mu[:], True, True, F32)
                rsb_ps = ps_bc.tile([128, TL], F32, tag="bc", name="rsb")
                _mm(nc, rsb_ps[:], ones_sb[0:1, :], rsig[:], True, True, F32)
                mub = spool.tile([128, TL], F32, tag="mubsb", name="mubsb",
                                 bufs=1)
                nc.vector.tensor_copy(mub[:], mub_ps[:])
                rsb = spool.tile([128, TL], F32, tag="rsbsb", name="rsbsb",
                                 bufs=1)
                nc.vector.tensor_copy(rsb[:], rsb_ps[:])
                for j in range(DT):
                    t1 = spool.tile([128, TL], F32, tag="lnt1", name="lnt1",
                                    bufs=2)
                    nc.vector.tensor_tensor(t1[:], xtiles[j][:], mub[:],
                                            ALU.subtract)
                    t2 = spool.tile([128, TL], F32, tag="lnt2", name="lnt2",
                                    bufs=2)
                    nc.vector.tensor_tensor(t2[:], t1[:], rsb[:], ALU.mult)
                    nc.vector.tensor_scalar(wslice(j), t2[:], g_sb[:, j:j + 1],
                                            b_sb[:, j:j + 1], ALU.mult,
                                            ALU.add)

            qT_sb = [ppool.tile([128, TL], F8, tag=f"qT{m}", name=f"qT{m}")
                     for m in range(DT)]

            with tc.tile_pool(name="st1", bufs=2) as s1pool:
                stat1 = ps_ao.tile([33, TL], F32, tag="ao", name="lnstat1")
                for j in range(DT):
                    ln_stats_j(stat1, xT_sb[j], j)
                layernorm_cm(xT_sb, ln1g_sb, ln1b_sb,
                             lambda j: x1T_p[j // 2][:, j % 2, :],
                             stat=stat1)

                def qk_slab(m):
                    # one 128-out-ch slab of the q/k GEMM (fp8)
                    slab = wpool.tile([128, DT * 128], F8, tag="qkslab",
                                      name="qkslab", bufs=3)
                    nc.sync.dma_start(
                        slab[:], wqk_d[:, m * 1024:(m + 1) * 1024])
                    sv = slab[:].rearrange("p (k c) -> p k c", c=128)
                    ps = ps_big.tile([128, TL], F32, tag="big", name="qk")
                    for u in range(DT // 2):
                        _mmdr(nc, ps[:], sv[:, 2 * u:2 * u + 2, :],
                              x1T_v[:, 2 * u:2 * u + 2, :],
                              u == 0, u == DT // 2 - 1)
                    if m < DT:
                        nc.vector.tensor_copy(qT_sb[m][:], ps[:])
                    else:
                        ksb = s1pool.tile([128, TL], F8, tag="kevac",
                                          name="kevac", bufs=2)
                        nc.vector.tensor_copy(ksb[:], ps[:])
                        mk = m - DT
                        nc.sync.dma_start(
                            kin_k(mk // 2)[(mk % 2) * 128:(mk % 2 + 1) * 128,
                                           :], ksb[:])

                wv_sb = [None, None]

                def v_quarter(qq):
                    # 256 v-channels (2 pairs), TM orientation. pad layout
                    # [q(2), hh(2), mt(4), 80]: col 64 = ones so the ao
                    # matmul also accumulates softmax denominators.
                    nn, iq = qq // 2, qq % 2
                    if iq == 0:
                        wv_sb[nn] = s1pool.tile([128, DT * 512], F8,
                                                tag=f"wv{nn}", name="wv",
                                                bufs=1)
                        nc.gpsimd.dma_start(
                            wv_sb[nn][:],
                            wv_d[:, nn * 4096:(nn + 1) * 4096])
                    wv_v = wv_sb[nn][:].rearrange("p (k c) -> p k c", c=512)
                    vp = s1pool.tile([128, 2 * 640], F8, tag="vpad",
                                     name="vpad", bufs=2)
                    nc.vector.memset(vp[:], 1.0)
                    dst = vp[:].rearrange("p (q hh m c) -> p q hh m c",
                                          hh=2, m=NT, c=80)
                    for mt in range(NT):         # 4 token Mtiles
                        ps = ps_big.tile([128, 256], F32, tag="big",
                                         name="vps")
                        for kk in range(DT):
                            _mm(nc, ps[:],
                                x1T_v[:, kk, mt * 128:(mt + 1) * 128],
                                wv_v[:, kk, iq * 256:(iq + 1) * 256],
                                kk == 0, kk == DT - 1)
                        nc.vector.tensor_copy(
                            dst[:, :, :, mt, 0:64],
                            ps[:].rearrange("p (q hh c) -> p q hh c",
                                            hh=2, c=64))
                    nc.sync.dma_start(kin_v(qq), vp[:])

                def ag(src, dst):
                    nc.gpsimd.collective_compute(
                        "AllGather", ALU.bypass, replica_groups=rg_kv,
                        ins=[src[:].opt()], outs=[dst[:].opt()])

                # per quarter: 2 k slabs + v quarter -> packed AG; then q
                for qq in range(4):
                    qk_slab(DT + 2 * qq)
                    qk_slab(DT + 2 * qq + 1)
                    v_quarter(qq)
                    ag(kv_in[qq], kv_out[qq])
                for m in range(DT):
                    qk_slab(m)

            # ---------- preload all MoE first-layer weights (1 DMA) -------
            we1_all = cpool.tile([128, 8 * DT * 128], F8, tag="we1a",
                                 name="we1a")
            nc.gpsimd.dma_start(we1_all[:], we1_d[:])
            we1_v = we1_all[:].rearrange("p (m k c) -> p m k c", k=DT, c=128)

            # ---------- attention (2-head interleave, fp8, DR ao) ----------
            with (
                tc.tile_pool(name="attn", bufs=2) as apool,
                tc.tile_pool(name="vsb", bufs=2) as vpool,
                tc.tile_pool(name="ssb", bufs=4) as spool_s,
            ):
                for p in range(DT):              # head pair
                    hf, pq = p // 2, p % 2       # kv quarter, pair in qtr
                    kp = []
                    vt = []
                    for r in range(GRP):
                        kt_ = apool.tile([128, TL], F8, tag=f"kp{r}",
                                         name=f"kp{r}", bufs=3)
                        nc.sync.dma_start(
                            kt_[:],
                            kout_k(hf, r)[pq * 128:(pq + 1) * 128, :])
                        kp.append(kt_)
                        vt_ = vpool.tile([128, 640], F8, tag=f"vt{r}",
                                         name=f"vt{r}", bufs=3)
                        nc.sync.dma_start(
                            vt_[:],
                            kout_v(hf, r)[:, pq * 640:(pq + 1) * 640])
                        vt.append(vt_)
                    ao_ps = [ps_ao.tile([66, TL], F32, tag="ao",
                                        name=f"ao{hh}") for hh in range(2)]
                    for beat in range(8):        # 2 key tiles per beat
                        for hh in range(2):
                            po = 64 * hh
                            s_sb = spool_s.tile([128, 2 * TL], F8,
                                                tag="ssb", name="ssb")
                            s2 = ps_big.tile([128, 2 * TL], F32, tag="big",
                                             name="s2")
                            for u in range(2):
                                kt = 2 * beat + u
                                r, cc = kt // 4, kt % 4
                                _mm(nc, s2[:, u * TL:(u + 1) * TL],
                                    kp[r][po:po + 64,
                                          cc * 128:(cc + 1) * 128],
                                    qT_sb[p][po:po + 64, :], True, True)
                            nc.scalar.activation(s_sb[:], s2[:],
                                                 AF.Exp, scale=SEXP)
                            r, cc = (2 * beat) // 4, (2 * beat) % 4
                            vtv = vt[r][:].rearrange(
                                "p (hh m c) -> p hh m c", hh=2, c=80)
                            _mmdr(nc, ao_ps[hh][:],
                                  vtv[:, hh, cc:cc + 2, 0:66],
                                  s_sb[:].rearrange("p (u t) -> p u t",
                                                    t=TL),
                                  beat == 0, beat == 7)
                    for hh in range(2):
                        po = 64 * hh
                        dsb = spool_s.tile([1, TL], F32, tag="densb",
                                           name="densb", bufs=1)
                        nc.vector.tensor_copy(dsb[:], ao_ps[hh][64:65, :])
                        recip = spool_s.tile([1, TL], F32, tag="recip",
                                             name="recip", bufs=1)
                        nc.vector.reciprocal_approx_fast(
                            out=recip[:], in_=dsb[:])
                        recb = spool_s.tile([1, TL], BF16, tag="recb",
                                            name="recb", bufs=1)
                        nc.vector.tensor_copy(recb[:], recip[:])
                        bc_ps = ps_bc.tile([64, TL], F32, tag="bc",
                                           name="aobc")
                        _mm(nc, bc_ps[:], c116_sb, recb[:],
                            True, True)
                        bc_sb = spool_s.tile([64, TL], BF16, tag="aobcsb",
                                             name="aobcsb", bufs=2)
                        nc.vector.tensor_copy(bc_sb[:], bc_ps[:])
                        nc.vector.tensor_tensor(
                            aoT_p[p // 2][po:po + 64, p % 2, :],
                            ao_ps[hh][0:64, :], bc_sb[:], ALU.mult)

                # ---------- proj + residual ----------
                stat2 = ps_ao.tile([33, TL], F32, tag="ao", name="lnstat2")
                xres = []
                for m in range(DT):
                    slab = wpool.tile([128, DT * 128], F8, tag="qkslab",
                                      name="projslab", bufs=3)
                    nc.sync.dma_start(
                        slab[:], wproj_d[:, m * 1024:(m + 1) * 1024])
                    sv = slab[:].rearrange("p (k c) -> p k c", c=128)
                    ps = ps_big.tile([128, TL], F32, tag="big", name="proj")
                    for u in range(DT // 2):
                        _mmdr(nc, ps[:], sv[:, 2 * u:2 * u + 2, :],
                              aoT_v[:, 2 * u:2 * u + 2, :],
                              u == 0, u == DT // 2 - 1)
                    pd = spool.tile([128, TL], F32, tag="projdq",
                                    name="projdq", bufs=2)
                    nc.scalar.activation(pd[:], ps[:], AF.Copy,
                                         scale=1.0 / (WS * WS))
                    xr = ppool.tile([128, TL], BF16, tag=f"xres{m}",
                                    name=f"xres{m}")
                    nc.vector.scalar_tensor_tensor(
                        xr[:], pd[:], bproj_sb[:, m:m + 1], xT_sb[m][:],
                        ALU.add, ALU.add)
                    xres.append(xr)
                    ln_stats_j(stat2, xr, m)

            # ---------- LN2 (bf16 out + fp8 copy) ----------
            x2T = [ppool.tile([128, TL], BF16, tag=f"x2T{j}", name=f"x2T{j}")
                   for j in range(DT)]
            layernorm_cm(xres, ln2g_sb, ln2b_sb,
                         lambda j: x2T[j][:], stat=stat2)

            # ---------- router ----------
            rt_ps = ps_ao.tile([32 + E, TL], F32, tag="ao", name="rt")
            logit_ps = rt_ps[0:E, :]
            nlin_ps = rt_ps[32:32 + E, :]
            for j in range(DT):
                _mm(nc, logit_ps, wroute_sb[:, j * E:(j + 1) * E],
                    x2T[j][:], j == 0, j == DT - 1)
            for j in range(DT):
                _mm(nc, nlin_ps, wnoise_sb[:, j * E:(j + 1) * E],
                    x2T[j][:], j == 0, j == DT - 1)
            logits = spool.tile([E, TL], F32, tag="logits", name="logits",
                                bufs=1)
            nc.vector.tensor_scalar(logits[:], logit_ps,
                                    broute_sb[:, 0:1], None, ALU.add)
            spe = spool.tile([E, TL], BF16, tag="softpe", name="softpe",
                             bufs=1)
            nc.scalar.activation(spe[:], nlin_ps, AF.Exp,
                                 bias=bnoise_sb[:, 0:1])
            spe1 = spool.tile([E, TL], BF16, tag="softpe1", name="softpe1",
                              bufs=1)
            nc.vector.tensor_scalar_add(spe1[:], spe[:], 1.0)
            sp = spool.tile([E, TL], BF16, tag="softp", name="softp",
                            bufs=1)
            nc.scalar.activation(sp[:], spe1[:], AF.Ln)
            nsp = spool.tile([E, TL], BF16, tag="nsp", name="nsp", bufs=1)
            nc.vector.tensor_tensor(nsp[:], noiseT_sb, sp[:], ALU.mult)
            noisy_cm = spool.tile([E, TL], F32, tag="noisycm", name="noisycm",
                                  bufs=1)
            nc.vector.tensor_tensor(noisy_cm[:], nsp[:], logits[:], ALU.add)

            # ---------- top-2 gates (TM) ----------
            noisy8 = ppool.tile([128, 8 * NT], F32, tag="noisy8",
                                name="noisy8")
            nc.vector.memset(noisy8[:], -1e30)
            m8 = ppool.tile([128, 8 * NT], F32, tag="m8", name="m8")
            gate = ppool.tile([128, E * NT], F32, tag="gate", name="gate")
            mask = ppool.tile([128, E * NT], F32, tag="mask", name="mask")
            geT = ppool.tile([E, TL], BF16, tag="geT", name="geT")
            cnt_sb = ppool.tile([1, NT * E], F32, tag="cntsb", name="cntsb")
            for j in range(NT):
                tr_ps = ps_bc.tile([128, E], F32, tag="bc", name="ntr")
                nc.tensor.matmul(tr_ps[:],
                                 noisy_cm[:, j * 128:(j + 1) * 128],
                                 eye_sb[0:E, 0:E], is_transpose=True,
                                 start=True, stop=True)
                nc.vector.tensor_copy(noisy8[:, 8 * j:8 * j + E], tr_ps[:])
            dv = spool.tile([128, NT], F32, tag="dv", name="dv")
            for j in range(NT):
                nc.vector.max(m8[:, 8 * j:8 * j + 8],
                              noisy8[:, 8 * j:8 * j + 8])
                nc.vector.tensor_tensor(dv[:, j:j + 1],
                                        m8[:, 8 * j + 1:8 * j + 2],
                                        m8[:, 8 * j:8 * j + 1],
                                        ALU.subtract)
            p2a = spool.tile([128, NT], F32, tag="p2a", name="p2a")
            nc.scalar.activation(p2a[:], dv[:], AF.Exp)
            dden = spool.tile([128, NT], F32, tag="dden", name="dden")
            nc.vector.tensor_scalar_add(dden[:], p2a[:], 1.0)
            rda = spool.tile([128, NT], F32, tag="rda", name="rda")
            nc.vector.reciprocal(rda[:], dden[:])
            for j in range(NT):
                nm = noisy8[:, 8 * j:8 * j + E]
                v1 = m8[:, 8 * j:8 * j + 1]
                v2 = m8[:, 8 * j + 1:8 * j + 2]
                oh1 = spool.tile([128, E], F32, tag="oh1", name="oh1")
                nc.vector.tensor_scalar(oh1[:], nm, v1, None, ALU.is_ge)
                msk = mask[:, E * j:E * (j + 1)]
                nc.vector.tensor_scalar(msk, nm, v2, None, ALU.is_ge)
                oh2 = spool.tile([128, E], F32, tag="oh2", name="oh2")
                nc.vector.tensor_tensor(oh2[:], msk, oh1[:], ALU.subtract)
                gnum = spool.tile([128, E], F32, tag="gnum", name="gnum")
                nc.vector.tensor_scalar(gnum[:], oh2[:], p2a[:, j:j + 1],
                                        None, ALU.mult)
                gnum2 = spool.tile([128, E], F32, tag="gnum2", name="gnum2")
                nc.vector.tensor_tensor(gnum2[:], gnum[:], oh1[:], ALU.add)
                nc.vector.tensor_scalar(gate[:, E * j:E * (j + 1)],
                                        gnum2[:], rda[:, j:j + 1], None,
                                        ALU.mult)
                cps = ps_bc.tile([1, E], F32, tag="bc", name="cnt")
                _mm(nc, cps[:], ones_sb[:, 0:1], msk, True, True, F32)
                nc.vector.tensor_copy(cnt_sb[0:1, E * j:E * (j + 1)], cps[:])

            # total counts -> all-gather
            tot = spool.tile([1, E], F32, tag="cnttot", name="cnttot",
                             bufs=1)
            nc.vector.tensor_tensor(tot[:], cnt_sb[0:1, 0:E],
                                    cnt_sb[0:1, E:2 * E], ALU.add)
            nc.vector.tensor_tensor(tot[:], tot[:], cnt_sb[0:1, 2 * E:3 * E],
                                    ALU.add)
            nc.vector.tensor_tensor(tot[:], tot[:], cnt_sb[0:1, 3 * E:4 * E],
                                    ALU.add)
            nc.sync.dma_start(cnt_in[:], tot[:])

            # ---------- MLP hidden + MoE hidden (overlaps counts AG) ------
            for j in range(DT):
                nc.scalar.activation(x2T8_p[j // 2][:, j % 2, :],
                                     x2T[j][:], AF.Copy)
            Hmoe = []
            for me in range(2 * E):
                ps = ps_big.tile([128, TL], F32, tag="big", name="hmoe")
                for u in range(DT // 2):
                    _mmdr(nc, ps[:], we1_v[:, me, 2 * u:2 * u + 2, :],
                          x2T8_v[:, 2 * u:2 * u + 2, :],
                          u == 0, u == DT // 2 - 1)
                hs = ppool.tile([128, TL], BF16, tag=f"hmoe{me}",
                                name=f"hmoe{me}")
                nc.scalar.activation(
                    hs[:], ps[:], AF.Gelu, scale=1.0 / WS,
                    bias=be1_sb[:, me:me + 1])
                Hmoe.append(hs)

            Hm_sb = []
            for m in range(MLPH // 128):
                slab = wpool.tile([128, DT * 128], BF16, tag="m1slab",
                                  name="m1slab", bufs=6)
                q1 = nc.sync if m % 2 == 0 else nc.gpsimd
                q1.dma_start(
                    slab[:], wmlp1_d[:, m * 1024:(m + 1) * 1024])
                ps = ps_big.tile([128, TL], F32, tag="big", name="hm")
                for kk in range(DT):
                    _mm(nc, ps[:], slab[:, kk * 128:(kk + 1) * 128],
                        x2T[kk][:], kk == 0, kk == DT - 1)
                hm = ppool.tile([128, TL], BF16, tag=f"hm{m}", name=f"hm{m}")
                nc.scalar.activation(hm[:], ps[:], AF.Gelu,
                                     bias=bmlp1_sb[:, m:m + 1])
                Hm_sb.append(hm)
            nc.gpsimd.collective_compute(
                "AllGather", ALU.bypass, replica_groups=rg_all,
                ins=[cnt_in[:].opt()], outs=[cnt_out[:].opt()])

            # ---------- preout: mlp2 GEMM + bias + residual (cnt-free) --
            preout = []
            for m in range(DT):
                slab2 = wpool.tile([128, 32 * 128], BF16, tag="outslab",
                                   name="outslab")
                nc.sync.dma_start(
                    slab2[:], wm2_d[:, m * 4096:(m + 1) * 4096])
                ps = ps_big.tile([128, TL], F32, tag="big", name="pre")
                for kk in range(MLPH // 128):
                    _mm(nc, ps[:], slab2[:, kk * 128:(kk + 1) * 128],
                        Hm_sb[kk][:], kk == 0, kk == MLPH // 128 - 1)
                nc.vector.scalar_tensor_tensor(
                    xres[m][:], ps[:], bmlp2_sb[:, m:m + 1], xres[m][:],
                    ALU.add, ALU.add)
                preout.append(xres[m])

            # ---------- ranks / keep / gate_eff ----------
            cntg = spool.tile([NC, E], F32, tag="cntg", name="cntg", bufs=1)
            nc.sync.dma_start(cntg[:], cnt_out[:])
            off_ps = ps_bc.tile([1, E], F32, tag="bc", name="off")
            _mm(nc, off_ps[:], wpfx_sb, cntg[:], True, True, F32)
            car = spool.tile([1, E * NT], F32, tag="car", name="car", bufs=1)
            nc.vector.tensor_copy(car[:, 0:E], off_ps[:])
            for j in range(1, NT):
                nc.vector.tensor_tensor(car[:, E * j:E * (j + 1)],
                                        car[:, E * (j - 1):E * j],
                                        cnt_sb[0:1, E * (j - 1):E * j],
                                        ALU.add)
            ge_tm = ppool.tile([128, E * NT], F32, tag="getm", name="getm")
            for j in range(NT):
                rk_ps = ps_bc.tile([128, E], F32, tag="bc", name="rank")
                _mm(nc, rk_ps[:], utri_sb,
                    mask[:, E * j:E * (j + 1)], True, False, F32)
                _mm(nc, rk_ps[:], ones_sb[0:1, :],
                    car[:, E * j:E * (j + 1)], False, True, F32)
                keep = spool.tile([128, E], F32, tag="keep", name="keep")
                nc.vector.tensor_scalar(keep[:], rk_ps[:], float(CAP), None,
                                        ALU.is_lt)
                nc.vector.tensor_tensor(ge_tm[:, E * j:E * (j + 1)],
                                        gate[:, E * j:E * (j + 1)],
                                        keep[:], ALU.mult)
            for j in range(NT):
                tr_ps = ps_bc.tile([E, 128], F32, tag="bc", name="getr")
                nc.tensor.matmul(tr_ps[:], ge_tm[:, E * j:E * (j + 1)],
                                 eye_sb, is_transpose=True,
                                 start=True, stop=True)
                nc.vector.tensor_copy(geT[:, j * 128:(j + 1) * 128], tr_ps[:])

            # gate the MoE hidden
            for e in range(E):
                bc_ps = ps_bc.tile([128, TL], F32, tag="bc", name="gbc")
                _mm(nc, bc_ps[:], gsel_sb[:, e * 128:(e + 1) * 128],
                    geT[:], True, True)
                bc_sb = spool.tile([128, TL], BF16, tag="gbcsb", name="gbcsb",
                                   bufs=2)
                nc.vector.tensor_copy(bc_sb[:], bc_ps[:])
                for hmi in range(MOEH // 128):
                    me = 2 * e + hmi
                    nc.vector.tensor_tensor(Hg_p[me // 2][:, me % 2, :],
                                            Hmoe[me][:],
                                            bc_sb[:], ALU.mult)

            # ---------- output GEMM: moe(DR fp8) + be2, add preout --------
            for m in range(DT):
                slab8 = wpool.tile([128, 8 * 128], F8, tag="out8",
                                   name="out8")
                nc.gpsimd.dma_start(
                    slab8[:], wout8_d[:, m * 1024:(m + 1) * 1024])
                sv8 = slab8[:].rearrange("p (k c) -> p k c", c=128)
                ps = ps_big.tile([128, TL], F32, tag="big", name="out")
                for u in range(4):           # we2 DR pairs
                    _mmdr(nc, ps[:], sv8[:, 2 * u:2 * u + 2, :],
                          Hg_v[:, 2 * u:2 * u + 2, :], u == 0, False)
                _mm(nc, ps[:], be2_sb[:, m * 128:(m + 1) * 128],
                    geT[:], False, True)
                o = spool.tile([128, TL], F32, tag="outsb", name="outsb",
                               bufs=2)
                nc.vector.tensor_tensor(o[:], ps[:], preout[m][:], ALU.add)
                nc.sync.dma_start(out_d[m * 128:(m + 1) * 128, :], o[:])

    nc.compile()
    return nc


def _tile_lhst(w, n_k, n_m):
    # w: [n_k*128, n_m*128] -> [128, n_m, n_k, 128] -> [128, n_m*n_k*128]
    kdim, mdim = w.shape
    return np.ascontiguousarray(
        w.reshape(n_k, 128, n_m, 128).transpose(1, 2, 0, 3)
        .reshape(128, n_m * n_k * 128))


def _prep_inputs(inputs):
    f32 = lambda a: np.ascontiguousarray(np.asarray(a, np.float32))
    bf = lambda a: np.ascontiguousarray(
        np.asarray(a, np.float32).astype(ml_dtypes.bfloat16))
    f8 = lambda a, s=1.0: np.ascontiguousarray(
        (np.asarray(a, np.float32) * s).astype(ml_dtypes.float8_e4m3))
    x = f32(inputs["x"]).reshape(T, D)
    noise = f32(inputs["noise"]).reshape(T, E)
    w_qkv = np.asarray(inputs["w_qkv"], np.float32)
    wqkT = w_qkv[:2 * D].T                       # [D, 2048]
    wvT = w_qkv[2 * D:].T                        # [D, D]
    wprojT = np.asarray(inputs["w_proj"], np.float32).T
    we1 = np.asarray(inputs["we1"], np.float32)  # [E, D, MOEH]
    we2 = np.asarray(inputs["we2"], np.float32)  # [E, MOEH, D]
    wmlp1 = np.asarray(inputs["w_mlp1"], np.float32)   # [D, MLPH]
    wmlp2 = np.asarray(inputs["w_mlp2"], np.float32)   # [MLPH, D]

    # we1 slabs: m-index = e*2+hmi over [D, 256] each
    we1_flat = np.concatenate([we1[e] for e in range(E)], 1)  # [D, E*MOEH]
    # wout8: per m, 8 we2 tiles (e,hmi); wm2: per m, 32 wmlp2 tiles
    we2_l = we2.reshape(E, 2, 128, DT, 128).transpose(2, 3, 0, 1, 4) \
        .reshape(128, DT * 8 * 128)
    wm2_l = wmlp2.reshape(32, 128, DT, 128).transpose(1, 2, 0, 3) \
        .reshape(128, DT * 32 * 128)
    # wv: [128, nn, kk, 512]
    wv_l = wvT.reshape(DT, 128, 2, 512).transpose(1, 2, 0, 3) \
        .reshape(128, 2 * DT * 512)

    cols = lambda a, n: np.asarray(a, np.float32).reshape(n, 128).T
    cf32 = np.concatenate([
        cols(inputs["ln1_g"], DT), cols(inputs["ln1_b"], DT),
        cols(inputs["ln2_g"], DT), cols(inputs["ln2_b"], DT),
        cols(inputs["b_proj"], DT), cols(inputs["be1"], DT),
        cols(inputs["b_mlp1"], 32), cols(inputs["b_mlp2"], DT),
        np.ones((128, 128), np.float32),
        np.eye(128, dtype=np.float32),
        np.triu(np.ones((128, 128), np.float32), 1),
    ], 1)
    cbf = np.concatenate([
        np.ones((128, 64), np.float32),
        np.asarray(inputs["w_route"], np.float32).reshape(DT, 128, E)
        .transpose(1, 0, 2).reshape(128, DT * E),
        np.asarray(inputs["w_noise"], np.float32).reshape(DT, 128, E)
        .transpose(1, 0, 2).reshape(128, DT * E),
    ], 1)
    gsel = np.repeat(np.eye(E, dtype=np.float32), 128, 1)

    shared = dict(
        wqk_l=f8(_tile_lhst(wqkT, DT, 16), WS),
        wv_l=f8(wv_l, WS),
        wproj_l=f8(_tile_lhst(wprojT, DT, DT), WS),
        wmlp1_l=bf(_tile_lhst(wmlp1, DT, 32)),
        we1_l=f8(_tile_lhst(we1_flat, DT, 8), WS),
        wout8_l=f8(we2_l),
        wm2_l=bf(wm2_l),
        cf32=f32(cf32),
        cbf=bf(cbf),
    )
    in_maps = []
    for c in range(NC):
        m = dict(shared)
        m["xT"] = bf(x[c * TL:(c + 1) * TL].T)
        m["cb4"] = bf(np.concatenate([
            gsel, np.asarray(inputs["be2"], np.float32),
            noise[c * TL:(c + 1) * TL].T], 1))
        crow = np.zeros((NC, 3), np.float32)
        crow[:, 0] = (np.arange(NC) < c)
        crow[0:E, 1] = np.asarray(inputs["b_route"], np.float32)
        crow[0:E, 2] = np.asarray(inputs["b_noise"], np.float32)
        m["crow"] = crow
        in_maps.append(m)
    return in_maps


def _run(inputs, trace=False):
    if "nc" not in _cache:
        _cache["nc"] = _build()
    nc = _cache["nc"]
    in_maps = _prep_inputs(inputs)
    res = run_bass_kernel_spmd(nc, in_maps, core_ids=list(range(NC)),
                               trace=trace)
    _cache["last_res"] = res
    shards = [res.results[c]["out"] for c in range(NC)]   # each [D, TL]
    out = np.concatenate([np.asarray(s, np.float32).T for s in shards],
                         0).reshape(B, N, D)
    return out.astype(np.float32), res.exec_time_ns


def kernel(**inputs):
    out, _ = _run(inputs, trace=False)
    return out
